# revision 1
# baseline (speedup 1.0000x reference)
"""Causal attention block (B=2, S=2048, H=1024, 16 heads) on 8 NeuronCores.

Sharding: core c handles batch b = c // 4 and head-group g = c % 4
(4 heads = 256 qkv columns / w_out rows per core). Each core computes a
partial output y_partial = softmax(QK^T/sqrt(d)) V @ Wout_slice for its
heads; the host sums the 4 head-group partials per batch.

On-chip layout (per core):
  x^T   [H=1024, S=2048]  (host-transposed)   - h on partitions
  Q^T,K^T as two head-PAIR tiles [128, 2048]: partitions 0-63 head 2p,
        64-127 head 2p+1 (d on partitions)    - from matmul(W, x^T)
  S^T = K^T.T @ Q^T per (t-chunk 128, s-chunk 512), row-tiled 2 heads
        concurrently on the PE (K=64 each)
  softmax without max-subtraction (scores are O(10), exp is safe in f32);
        causal masking via additive -1e38 mask (built on-chip with
        affine_select) added into the PSUM triangular band before a single
        exp per block; fully-masked columns are never computed (narrower
        matmuls / exps; PSUM accumulation is per-element so this is exact)
  PV: out^T accumulation with V augmented by a ones column, which makes
        the denominator Z land in an extra PSUM row for free
  normalize: DVE reciprocal -> PE K=1 outer-product broadcast -> DVE mul
        (gpsimd partition_broadcast and 1-partition custom-DVE ops are
        broken on this hardware; DMA rejects partition-step-0 APs)
  out-proj: y = V~^T.T @ Wout per s-tile, accumulated over 2 pairs;
        j=3 borrows the idle qkv PSUM banks for deeper S^T/out-proj
        pipelining in the ACT-bound causal tail

All matmuls use fp32r (4-byte, ~tf32 precision, 1 cycle/row at N>=256).
"""

import numpy as np
from contextlib import ExitStack

import concourse.bass as bass
import concourse.tile as tile
import concourse.mybir as mybir
from concourse import bacc
from concourse import bass_utils

F32 = mybir.dt.float32
F32R = mybir.dt.float32r
AF = mybir.ActivationFunctionType

B, S, H = 2, 2048, 1024
NH, DH = 16, 64
NCORES = 8
SC = 512            # s-chunk width
NSC = S // SC       # 4
NTC = S // 128      # 16 t-chunks
NHC = H // 128      # 8 h contraction chunks

_CACHE = {}


def _build():
    nc = bacc.Bacc("TRN2", target_bir_lowering=False, debug=False,
                   enable_asserts=False, num_devices=NCORES)
    xT = nc.dram_tensor("xT", [H, S], F32, kind="ExternalInput").ap()
    wq = nc.dram_tensor("wq", [H, 256], F32, kind="ExternalInput").ap()
    wk = nc.dram_tensor("wk", [H, 256], F32, kind="ExternalInput").ap()
    wv = nc.dram_tensor("wv", [H, 256], F32, kind="ExternalInput").ap()
    wo = nc.dram_tensor("wo", [256, H], F32, kind="ExternalInput").ap()
    vaug = nc.dram_tensor("vaug", [128, 130], F32, kind="ExternalInput").ap()
    ones = nc.dram_tensor("ones", [128, SC], F32, kind="ExternalInput").ap()
    y = nc.dram_tensor("y", [S, H], F32, kind="ExternalOutput").ap()

    with tile.TileContext(nc) as tc:
        with ExitStack() as ctx:
            pw = ctx.enter_context(tc.tile_pool(name="w", bufs=1))
            pxt = ctx.enter_context(tc.tile_pool(name="xt", bufs=2))
            pbig = ctx.enter_context(tc.tile_pool(name="big", bufs=1))
            import os as _os
            ppt = ctx.enter_context(tc.tile_pool(name="pt", bufs=int(_os.environ.get("KPT", "8"))))
            pzz = ctx.enter_context(tc.tile_pool(name="zz", bufs=3))
            pyo = ctx.enter_context(tc.tile_pool(name="yo", bufs=4))
            import os
            _b = os.environ.get("KBUFS", "2,3,2,1").split(",")
            bq, bs, bp, by = (int(v) for v in _b)  # PSUM banks: qkv/s/pv/y
            ps_qkv = ctx.enter_context(tc.tile_pool(name="psqkv", bufs=bq, space="PSUM"))
            ps_s = ctx.enter_context(tc.tile_pool(name="pss", bufs=bs, space="PSUM"))
            ps_pv = ctx.enter_context(tc.tile_pool(name="pspv", bufs=bp, space="PSUM"))
            ps_y = ctx.enter_context(tc.tile_pool(name="psy", bufs=by, space="PSUM"))

            # ---- weights & masks (scalar-engine DGE queue; sync queue
            #      carries the x^T / y traffic) ----
            def load_w_all(dram, nm):
                t = pw.tile([128, NHC * 256], F32R, tag=nm, name=nm)
                nc.scalar.dma_start(
                    t[:].rearrange("p (c n) -> p c n", c=NHC),
                    dram.rearrange("(c p) n -> p c n", p=128).bitcast(F32R))
                return [t[:, hc * 256:(hc + 1) * 256] for hc in range(NHC)]

            wq_t = load_w_all(wq, "wqa")
            wk_t = load_w_all(wk, "wka")
            # wk / masks / vaug / wo are loaded later (inside the j-loop)
            # so the x^T chunk transfers win shared HBM bandwidth first.
            wo_t, mask_t, wv_t = [], [], []

            # ---- persistent activations ----
            QT = [pbig.tile([128, S], F32R, tag=f"qt{p}", name=f"qt{p}") for p in range(2)]
            KT = [pbig.tile([128, S], F32R, tag=f"kt{p}", name=f"kt{p}") for p in range(2)]
            VT = [pbig.tile([128, S], F32R, tag=f"vt{p}", name=f"vt{p}") for p in range(2)]
            # V_aug per t-chunk, grouped per head pair (193 cols each):
            # even head-local: [V(64) | 1]         -> out rows 0..64, Z row 64
            # odd  head-local: [zeros(32) | 1 | zeros(31) | V] -> out rows 0..127
            #                  (base 0), Z row 32, V~ rows 64..127
            VA = [pbig.tile([128, 386], F32R, tag=f"va{t_}", name=f"va{t_}") for t_ in range(NTC)]

            for j in range(NSC):
                sj = slice(j * SC, (j + 1) * SC)
                # ---- load x^T column-block j (two 1 MB halves so the
                #      hc=0..3 accumulation can start while 4..7 streams) ----
                xt_all = pxt.tile([128, NHC * SC], F32R, tag="xt",
                                  name=f"xt{j}")
                xt_src = xT.rearrange("(c p) s -> p c s", p=128)[:, :, sj]
                xt_dst = xt_all[:].rearrange("p (c s) -> p c s", c=NHC)
                nsplit = 4 if j == 0 else 2
                step = NHC // nsplit
                for si in range(nsplit):
                    nc.sync.dma_start(
                        xt_dst[:, si * step:(si + 1) * step, :],
                        xt_src[:, si * step:(si + 1) * step, :].bitcast(F32R))
                xt_j = [xt_all[:, hc * SC:(hc + 1) * SC] for hc in range(NHC)]

                if j == 0:
                    wv_t = load_w_all(wv, "wva")
                    ones_t = pw.tile([128, SC], F32R, tag="ones")
                    nc.scalar.dma_start(ones_t[:], ones[:].bitcast(F32R))
                    vaug_sb = pw.tile([128, 130], F32R, tag="vaug")
                    nc.scalar.dma_start(vaug_sb[:], vaug[:].bitcast(F32R))
                    # additive causal masks built on-chip:
                    # mask_k[p, f] = 0 if f >= 128k + p else -1e38
                    for k4 in range(4):
                        mt = pw.tile([128, SC], F32, tag=f"mask{k4}",
                                     name=f"mask{k4}")
                        nc.gpsimd.affine_select(
                            mt[:], ones_t[:].bitcast(F32),
                            pattern=[[1, SC]], base=-128 * k4,
                            channel_multiplier=-1,
                            compare_op=mybir.AluOpType.is_ge, fill=-1.0e38)
                        nc.vector.tensor_scalar_sub(mt[:], mt[:], 1.0)
                        mask_t.append(mt)
                # ---- Q^T / K^T for s-chunk j ----
                for p in range(2):
                    for W, OUT in ((wq_t, QT), (wk_t, KT)):
                        ps = ps_qkv.tile([128, SC], F32, tag="qkv")
                        for hc in range(NHC):
                            nc.tensor.matmul(
                                ps[:], W[hc][:, p * 128:(p + 1) * 128],
                                xt_j[hc],
                                start=(hc == 0), stop=(hc == NHC - 1))
                        nc.vector.tensor_copy(OUT[p][:, sj], ps[:])

                # ---- V for t-chunks 4j..4j+3 ----
                for tci in range(4):
                    t_ = 4 * j + tci
                    ps = ps_qkv.tile([128, 256], F32, tag="qkv")
                    for hc in range(NHC):
                        nc.tensor.matmul(
                            ps[:],
                            xt_all[:, hc * SC + tci * 128:
                                   hc * SC + (tci + 1) * 128],
                            wv_t[hc], start=(hc == 0), stop=(hc == NHC - 1))
                    va3 = VA[t_][:].rearrange("p (g c) -> p g c", c=193)
                    psv3 = ps[:].rearrange("p (g c) -> p g c", c=128)
                    nc.vector.tensor_copy(va3[:, :, 0:64], psv3[:, :, 0:64])
                    nc.vector.tensor_copy(va3[:, :, 129:193], psv3[:, :, 64:128])
                    nc.vector.tensor_copy(
                        va3[:, :, 64:129],
                        vaug_sb[:].rearrange("p (g c) -> p g c", c=65))

                # ---- attention for s-chunk j ----
                ntc = 4 * j + 4
                for p in range(2):
                    pp = {}
                    for r in range(2):
                        pp[r] = ps_pv.tile([128, SC], F32, tag="pv", name=f"pv{p}_{r}")
                    for tcc in range(ntc):
                        # diagonal blocks only touch s-columns >= 128k
                        # (k = position within the diagonal 512x512 square);
                        # cols < 128k are fully masked and never computed.
                        if tcc >= 4 * j:
                            k = tcc - 4 * j
                            c0 = 128 * k          # valid col start
                            c1 = 128 * (k + 1)    # end of triangular band
                        else:
                            k, c0, c1 = None, 0, 0
                        # fp32r matmuls below 256 moving cols run at
                        # 4 cyc/row; widen the k=3 S^T matmul to 256 cols
                        # (extra cols land in psum but are never exp'd/read)
                        c0m = min(c0, SC - 256)
                        sjv = slice(j * SC + c0m, (j + 1) * SC)
                        pts = {}
                        for r in range(2):
                            pool_s = (ps_qkv if (j == 3 and (tcc + r) % 2 == 0)
                                      else ps_s)
                            ss = pool_s.tile([128, SC], F32,
                                             tag="qkv" if pool_s is ps_qkv
                                             else "s", name=f"ss{r}")
                            nc.tensor.matmul(
                                ss[:, c0m:SC],
                                KT[p][64 * r:64 * (r + 1),
                                      tcc * 128:(tcc + 1) * 128],
                                QT[p][64 * r:64 * (r + 1), sjv],
                                start=True, stop=True)
                            pt = ppt.tile([128, SC], F32R, tag="pt")
                            if k is not None:
                                # triangular band: add -1e38 mask, then one exp
                                nc.vector.tensor_add(ss[:, c0:c1],
                                                     ss[:, c0:c1],
                                                     mask_t[k][:, c0:c1])
                                nc.scalar.activation(pt[:, c0:SC],
                                                     ss[:, c0:SC], AF.Exp)
                            else:
                                nc.scalar.activation(pt[:], ss[:], AF.Exp)
                            pts[r] = pt
                        for r in range(2):
                            if r == 0:
                                out_sl = pp[r][0:65, c0:SC]
                                lhs_sl = VA[tcc][:, 193 * p:193 * p + 65]
                            else:
                                out_sl = pp[r][0:128, c0:SC]
                                lhs_sl = VA[tcc][:, 193 * p + 65:193 * p + 193]
                            nc.tensor.matmul(
                                out_sl, lhs_sl, pts[r][:, c0:SC],
                                start=(tcc == 0), stop=(tcc == ntc - 1))
                    # normalize: V~^T = PV / Z
                    # recip (DVE) -> PE outer-product broadcast -> copy -> mul
                    for r in range(2):
                        z_row = 64 if r == 0 else 32
                        zr = pzz.tile([65, SC], F32R, tag="zr")
                        with nc.allow_low_precision(reason="f32r recip feeds bcast matmul"):
                            nc.vector.reciprocal(
                                zr[z_row:z_row + 1, :], pp[r][z_row:z_row + 1, :])
                        rbp = ps_y.tile([128, SC], F32, tag="y",
                                        name=f"rbp{p}_{r}")
                        nc.tensor.matmul(rbp[:],
                                         ones_t[z_row:z_row + 1, 0:128],
                                         zr[z_row:z_row + 1, :],
                                         start=True, stop=True)
                        rb = pzz.tile([128, SC], F32, tag="rb")
                        if r == 0:
                            rb_sl, v_sl = rb[0:64, :], pp[r][0:64, :]
                        else:
                            rb_sl, v_sl = rb[64:128, :], pp[r][64:128, :]
                        nc.vector.tensor_copy(rb_sl, rbp[0:64, :] if r == 0
                                              else rbp[64:128, :])
                        if j == 3:
                            # 128-col slices so the tail out-proj can start
                            # on the first s-tile before the rest normalize
                            for q4 in range(4):
                                qs = slice(q4 * 128, (q4 + 1) * 128)
                                nc.vector.tensor_mul(
                                    VT[p][64 * r:64 * (r + 1),
                                          j * SC + q4 * 128:
                                          j * SC + (q4 + 1) * 128],
                                    v_sl[:, qs], rb_sl[:, qs])
                        else:
                            nc.vector.tensor_mul(
                                VT[p][64 * r:64 * (r + 1), sj], v_sl, rb_sl)

                # ---- out-projection for s-tiles in chunk j ----
                if j == 0:
                    for p in range(2):
                        t = pw.tile([128, H], F32R, tag=f"wo{p}",
                                    name=f"wo{p}")
                        nc.scalar.dma_start(
                            t[:], wo[p * 128:(p + 1) * 128, :].bitcast(F32R))
                        wo_t.append(t)
                for sti in range(4):
                    st = 4 * j + sti
                    ysb = pyo.tile([128, H], F32, tag="y", name=f"ysb{st}")
                    for n2 in range(2):
                        pool_y = ps_qkv if (j == 3 and n2 == 1) else ps_y
                        py_ = pool_y.tile([128, 512], F32,
                                          tag="qkv" if pool_y is ps_qkv
                                          else "y", name=f"py{sti}_{n2}")
                        for p in range(2):
                            nc.tensor.matmul(
                                py_[:], VT[p][:, st * 128:(st + 1) * 128],
                                wo_t[p][:, n2 * 512:(n2 + 1) * 512],
                                start=(p == 0), stop=(p == 1))
                        if j == 3 and n2 == 1:
                            # ACT is idle in the tail; run the second half
                            # there so DVE and ACT drain in parallel
                            nc.scalar.copy(
                                ysb[:, n2 * 512:(n2 + 1) * 512], py_[:])
                        else:
                            nc.vector.tensor_copy(
                                ysb[:, n2 * 512:(n2 + 1) * 512], py_[:])
                    if j == 3:
                        nc.sync.dma_start(
                            y[st * 128:(st + 1) * 128, 0:512], ysb[:, 0:512])
                        nc.sync.dma_start(
                            y[st * 128:(st + 1) * 128, 512:H], ysb[:, 512:H])
                    else:
                        nc.sync.dma_start(y[st * 128:(st + 1) * 128, :],
                                          ysb[:])
    nc.compile()
    return nc


def _masks():
    k = np.arange(4)[:, None, None]
    p = np.arange(128)[None, :, None]
    f = np.arange(SC)[None, None, :]
    return (f >= 128 * k + p).astype(np.float32)


def _in_maps(x, w_qkv, w_out):
    x = np.asarray(x, dtype=np.float32)
    w_qkv = np.asarray(w_qkv, dtype=np.float32)
    w_out = np.asarray(w_out, dtype=np.float32)
    vaug_const = np.zeros((128, 130), dtype=np.float32)
    vaug_const[:, 0] = 1.0      # even-head ones col (group col 64)
    vaug_const[:, 33] = 1.0     # odd-head ones col (group col 97)
    vaug_const[:, 65] = 1.0
    vaug_const[:, 98] = 1.0
    ones_const = np.ones((128, SC), dtype=np.float32)
    scale = np.float32(1.0 / np.sqrt(DH))
    in_maps = []
    for c in range(NCORES):
        b, g = divmod(c, 4)
        cols = slice(256 * g, 256 * (g + 1))
        in_maps.append({
            "xT": np.ascontiguousarray(x[b].T),
            "wq": np.ascontiguousarray(w_qkv[:, 0 * H:1 * H][:, cols]) * scale,
            "wk": np.ascontiguousarray(w_qkv[:, 1 * H:2 * H][:, cols]),
            "wv": np.ascontiguousarray(w_qkv[:, 2 * H:3 * H][:, cols]),
            "wo": np.ascontiguousarray(w_out[cols, :]),
            "vaug": vaug_const,
            "ones": ones_const,
        })
    return in_maps


TRACE = False
LAST_RESULTS = None


def kernel(x, w_qkv, w_out):
    global LAST_RESULTS
    if "nc" not in _CACHE:
        _CACHE["nc"] = _build()
    nc = _CACHE["nc"]
    in_maps = _in_maps(x, w_qkv, w_out)
    res = bass_utils.run_bass_kernel_spmd(
        nc, in_maps, core_ids=list(range(NCORES)), trace=TRACE)
    LAST_RESULTS = res
    y = np.zeros((B, S, H), dtype=np.float32)
    for c in range(NCORES):
        y[c // 4] += res.results[c]["y"]
    return y



# revision 11
# speedup vs baseline: 1.1958x; 1.1958x over previous
"""Causal attention block (B=2, S=2048, H=1024, 16 heads) on 8 NeuronCores.

Sharding: core c handles batch b = c // 4 and head-group g = c % 4
(4 heads = 256 qkv columns / w_out rows per core). Each core computes a
partial output y_partial = softmax(QK^T/sqrt(d)) V @ Wout_slice for its
heads; the host sums the 4 head-group partials per batch.

v4 design (bf16 compute, f32 PSUM):
  x^T [1024, 2048] bf16; Q^T,K^T head-pair tiles [128, 2048] bf16;
  V natural [t, d] per t-chunk (VA [128, 256] bf16).
  S^T per (head, t-chunk, s-chunk 512) -> exp on ACT -> pt bf16; diagonal
  triangle zeroed post-exp by gpsimd affine_select.
  PV flipped to [s, d]: pt block is the stationary operand, VA the 64-wide
  moving operand; softmax denominators via N=1 matmuls against a ones
  column (Z lands per-partition), normalize = per-partition scalar mul,
  then one PE transpose per head pair back to [d, s] for the out-proj.
  Global software pipeline: PV lags S/exp by one step, transposes by two;
  QKV chains for the next chunk are deadline-interleaved as PE filler;
  out-projections are deferred (half-H sub-units) into the ACT-bound tail.
  PSUM: one shared 5-buf tag for scores/chains/out-proj + 2 PV banks +
  1 bank for Z columns and transpose slots.
"""

import numpy as np
from collections import deque
from contextlib import ExitStack

import concourse.bass as bass
import concourse.tile as tile
import concourse.mybir as mybir
from concourse import bacc
from concourse import bass_utils

F32 = mybir.dt.float32
BF16 = mybir.dt.bfloat16
AF = mybir.ActivationFunctionType

B, S, H = 2, 2048, 1024
NH, DH = 16, 64
NCORES = 8
SC = 512            # s-chunk width
NSC = S // SC       # 4
NHC = H // 128      # 8 h contraction chunks

_CACHE = {}


def _gstep(j):
    return 2 * j * (j + 1)


def _build():
    nc = bacc.Bacc("TRN2", target_bir_lowering=False, debug=False,
                   enable_asserts=False, num_devices=NCORES)
    xT = nc.dram_tensor("xT", [H, S], BF16, kind="ExternalInput").ap()
    wq = nc.dram_tensor("wq", [H, 256], BF16, kind="ExternalInput").ap()
    wk = nc.dram_tensor("wk", [H, 256], BF16, kind="ExternalInput").ap()
    wv = nc.dram_tensor("wv", [H, 256], BF16, kind="ExternalInput").ap()
    wo = nc.dram_tensor("wo", [256, H], BF16, kind="ExternalInput").ap()
    aux = nc.dram_tensor("aux", [128, 132], BF16, kind="ExternalInput").ap()
    y = nc.dram_tensor("y", [S, H], BF16, kind="ExternalOutput").ap()
    import os
    KDBG = os.environ.get("KDBG", "0") == "1"
    if KDBG:
        dbg = {nm: nc.dram_tensor(nm, shp, dt, kind="ExternalOutput").ap()
               for nm, shp, dt in [
                   ("d_vt0", [128, S], BF16), ("d_vt1", [128, S], BF16),
                   ("d_qt0", [128, S], BF16), ("d_kt0", [128, S], BF16),
                   ("d_va0", [128, 256], BF16), ("d_zz0", [128, 16], F32),
                   ("d_v2_0", [128, 256], BF16), ("d_pt00", [128, SC], BF16),
                   ("d_pt01", [128, SC], BF16), ("d_pv1", [128, 64], F32),
                   ("d_v2_1", [128, 256], BF16),
               ]}

    with tile.TileContext(nc) as tc:
        with ExitStack() as ctx:
            pw = ctx.enter_context(tc.tile_pool(name="w", bufs=1))
            pxt = ctx.enter_context(tc.tile_pool(name="xt", bufs=2))
            pbig = ctx.enter_context(tc.tile_pool(name="big", bufs=1))
            ppt = ctx.enter_context(tc.tile_pool(name="pt", bufs=72))
            pzz = ctx.enter_context(tc.tile_pool(name="zz", bufs=2))
            pv2 = ctx.enter_context(tc.tile_pool(name="v2", bufs=3))
            pyo = ctx.enter_context(tc.tile_pool(name="yo", bufs=4))
            psum = ctx.enter_context(
                tc.tile_pool(name="psum", bufs=1, space="PSUM"))

            def s_tile(name):
                return psum.tile([128, SC], F32, tag="s", bufs=6, name=name)

            # ---- weights on the scalar (ACT) DGE queue, before any exp ----
            def load_w_all(dram, nm, nsplit=1):
                t = pw.tile([128, NHC * 256], BF16, tag=nm, name=nm)
                dst = t[:].rearrange("p (c n) -> p c n", c=NHC)
                src = dram.rearrange("(c p) n -> p c n", p=128)
                step = NHC // nsplit
                for si in range(nsplit):
                    nc.scalar.dma_start(
                        dst[:, si * step:(si + 1) * step, :],
                        src[:, si * step:(si + 1) * step, :])
                return [t[:, hc * 256:(hc + 1) * 256] for hc in range(NHC)]

            wq_t = load_w_all(wq, "wqa", nsplit=2)
            wk_t = load_w_all(wk, "wka")
            aux_t = pw.tile([128, 132], BF16, tag="aux")
            nc.scalar.dma_start(aux_t[:], aux[:])
            ident = aux_t[:, 0:128]
            ones1 = aux_t[:, 128:129]
            wv_t = load_w_all(wv, "wva")
            wo_t = []
            for p in range(2):
                t = pw.tile([128, H], BF16, tag=f"wo{p}", name=f"wo{p}")
                nc.scalar.dma_start(t[:], wo[p * 128:(p + 1) * 128, :])
                wo_t.append(t)

            # ---- persistent activations ----
            QT = [pbig.tile([128, S], BF16, tag=f"qt{p}", name=f"qt{p}")
                  for p in range(2)]
            KT = [pbig.tile([128, S], BF16, tag=f"kt{p}", name=f"kt{p}")
                  for p in range(2)]
            VT = [pbig.tile([128, S], BF16, tag=f"vt{p}", name=f"vt{p}")
                  for p in range(2)]
            VA = [pbig.tile([128, 256], BF16, tag=f"va{t_}", name=f"va{t_}")
                  for t_ in range(S // 128)]

            xt_tiles = [None] * NSC

            def load_xt(j, nsplit):
                xt_all = pxt.tile([128, NHC * SC], BF16, tag="xt",
                                  name=f"xt{j}")
                sj = slice(j * SC, (j + 1) * SC)
                xt_src = xT.rearrange("(c p) s -> p c s", p=128)[:, :, sj]
                xt_dst = xt_all[:].rearrange("p (c s) -> p c s", c=NHC)
                step = NHC // nsplit
                for si in range(nsplit):
                    nc.sync.dma_start(
                        xt_dst[:, si * step:(si + 1) * step, :],
                        xt_src[:, si * step:(si + 1) * step, :])
                xt_tiles[j] = xt_all

            # ---- QKV projection chains ----
            def qk_chain(j, W, OUT, p):
                def emit():
                    xt_all = xt_tiles[j]
                    sj = slice(j * SC, (j + 1) * SC)
                    ps = s_tile(f"qk{j}_{p}")
                    for hc in range(NHC):
                        nc.tensor.matmul(
                            ps[:], W[hc][:, p * 128:(p + 1) * 128],
                            xt_all[:, hc * SC:(hc + 1) * SC],
                            start=(hc == 0), stop=(hc == NHC - 1))
                    nc.vector.tensor_copy(OUT[p][:, sj], ps[:])
                return emit

            def v_chain(j, tci):
                def emit():
                    xt_all = xt_tiles[j]
                    t_ = 4 * j + tci
                    ps = s_tile(f"v{j}_{tci}")
                    for hc in range(NHC):
                        nc.tensor.matmul(
                            ps[:, 0:256],
                            xt_all[:, hc * SC + tci * 128:
                                   hc * SC + (tci + 1) * 128],
                            wv_t[hc], start=(hc == 0), stop=(hc == NHC - 1))
                    nc.vector.tensor_copy(VA[t_][:], ps[:, 0:256])
                return emit

            # ---- chunk-local state: pt tiles persist per chunk ----
            class ChunkCtx:
                def __init__(self, j):
                    self.pts = {}   # (tcc, h) -> pt tile
                    self.zz = pzz.tile([128, 16], F32, tag="zz",
                                       name=f"zz{j}")

            # ---- attention pieces ----
            def emit_S(cc, j, tcc, prs):
                k = tcc - 4 * j
                c0 = max(0, 128 * k)
                sjv = slice(j * SC + c0, (j + 1) * SC)
                for (p, r) in prs:
                    ss = s_tile(f"ss{tcc}_{p}{r}")
                    nc.tensor.matmul(
                        ss[:, c0:SC],
                        KT[p][64 * r:64 * (r + 1),
                              tcc * 128:(tcc + 1) * 128],
                        QT[p][64 * r:64 * (r + 1), sjv],
                        start=True, stop=True)
                    pt = ppt.tile([128, SC], BF16, tag="pt")
                    nc.scalar.activation(pt[:, c0:SC], ss[:, c0:SC], AF.Exp)
                    if k >= 0:
                        nc.gpsimd.affine_select(
                            pt[:, c0:c0 + 128], pt[:, c0:c0 + 128],
                            pattern=[[1, 128]], base=0,
                            channel_multiplier=-1,
                            compare_op=mybir.AluOpType.is_ge, fill=0.0)
                    cc.pts[(tcc, 2 * p + r)] = pt
                    if KDBG and j == 0 and tcc == 0 and p == 0 and r == 0:
                        nc.sync.dma_start(dbg["d_pt00"][:], pt[:])
                    if KDBG and j == 0 and tcc == 1 and p == 0 and r == 0:
                        nc.sync.dma_start(dbg["d_pt01"][:], pt[:])

            subs = deque()        # deferred out-projection sub-closures
            pe_extras = deque()   # deferred transpose closures

            def make_transpose(cc, j, sti, v2):
                st = 4 * j + sti

                def emit():
                    for p in range(2):
                        trt = s_tile(f"tr{st}_{p}")
                        trp = trt[:, 0:64].bitcast(BF16)
                        nc.tensor.transpose(
                            trp, v2[:, 128 * p:128 * (p + 1)], ident)
                        nc.vector.tensor_copy(
                            VT[p][:, st * 128:(st + 1) * 128], trp)
                    ysb = pyo.tile([128, H], BF16, tag="y", name=f"ysb{st}")
                    for n2 in range(2):
                        subs.append(make_sub(st, n2, ysb))
                return emit

            def make_sub(st, n2, ysb):
                def emit():
                    py_ = s_tile(f"py{st}_{n2}")
                    for p in range(2):
                        nc.tensor.matmul(
                            py_[:], VT[p][:, st * 128:(st + 1) * 128],
                            wo_t[p][:, n2 * 512:(n2 + 1) * 512],
                            start=(p == 0), stop=(p == 1))
                    nc.vector.tensor_copy(
                        ysb[:, n2 * 512:(n2 + 1) * 512], py_[:])
                    nc.sync.dma_start(
                        y[st * 128:(st + 1) * 128, n2 * 512:(n2 + 1) * 512],
                        ysb[:, n2 * 512:(n2 + 1) * 512])
                return emit

            def emit_PV(cc, j, sti):
                # one unbroken accumulation group per (s-tile, head):
                # PV over tcc=0..st, then z over tcc=0..st, sequentially
                # through one psum bank (one open group per bank at a time)
                st = 4 * j + sti
                bank = psum.tile([128, SC], F32, tag="pv", bufs=2,
                                 name=f"pv{st}")
                for h in range(4):
                    for tcc in range(st + 1):
                        ptsl = cc.pts[(tcc, h)][:, sti * 128:(sti + 1) * 128]
                        nc.tensor.matmul(
                            bank[:, 64 * h:64 * (h + 1)], ptsl,
                            VA[tcc][:, 64 * h:64 * (h + 1)],
                            start=(tcc == 0), stop=(tcc == st))
                    for tcc in range(st + 1):
                        ptsl = cc.pts[(tcc, h)][:, sti * 128:(sti + 1) * 128]
                        nc.tensor.matmul(
                            bank[:, 256 + h:257 + h], ptsl, ones1,
                            start=(tcc == 0), stop=(tcc == st))
                nc.vector.reciprocal(
                    cc.zz[:, 4 * sti:4 * sti + 4], bank[:, 256:260])
                v2 = pv2.tile([128, 256], BF16, tag="v2", name=f"v2_{st}")
                for h in range(4):
                    nc.vector.tensor_scalar_mul(
                        v2[:, 64 * h:64 * (h + 1)],
                        bank[:, 64 * h:64 * (h + 1)],
                        cc.zz[:, 4 * sti + h:4 * sti + h + 1])
                if KDBG and st == 0:
                    nc.sync.dma_start(dbg["d_v2_0"][:], v2[:])
                if KDBG and st == 1:
                    nc.sync.dma_start(dbg["d_zz0"][:], cc.zz[:])
                    nc.sync.dma_start(dbg["d_v2_1"][:], v2[:])
                    dsb = pw.tile([128, 64], F32, tag="dsb")
                    nc.vector.tensor_copy(dsb[:], bank[:, 0:64])
                    nc.sync.dma_start(dbg["d_pv1"][:], dsb[:])
                pe_extras.append(make_transpose(cc, j, sti, v2))

            # ---- global schedule ----
            chains = deque()   # (deadline_step, emit_fn)

            load_xt(0, 4)
            qk_chain(0, wq_t, QT, 0)()
            qk_chain(0, wk_t, KT, 0)()
            qk_chain(0, wq_t, QT, 1)()
            qk_chain(0, wk_t, KT, 1)()
            for tci in range(4):
                chains.append((tci + 1, v_chain(0, tci)))

            pending = None
            g = 0
            for j in range(NSC):
                ntc = 4 * j + 4
                cc = ChunkCtx(j)
                if j + 1 < NSC:
                    load_xt(j + 1, 2)
                    g1 = _gstep(j + 1)
                    for p in range(2):
                        chains.append((g1, qk_chain(j + 1, wq_t, QT, p)))
                    for p in range(2):
                        chains.append((g1 + 4 * (j + 1),
                                       qk_chain(j + 1, wk_t, KT, p)))
                    for tci in range(4):
                        chains.append((g1 + 4 * (j + 1) + tci,
                                       v_chain(j + 1, tci)))
                for tcc in range(ntc):
                    k = tcc - 4 * j
                    emit_S(cc, j, tcc, [(0, 0), (0, 1), (1, 0)])
                    while pe_extras:
                        pe_extras.popleft()()
                    if pending is not None:
                        emit_PV(*pending)
                        pending = None
                    emit_S(cc, j, tcc, [(1, 1)])
                    if k >= 0:
                        pending = (cc, j, k)
                    # non-S psum users this step: <= 1 chain + <= 1 sub
                    popped = 0
                    while chains and (chains[0][0] <= g + 1 or popped == 0):
                        chains.popleft()[1]()
                        popped += 1
                        if popped >= 2 and not (chains
                                                and chains[0][0] <= g + 1):
                            break
                    if subs and (j == 3 or len(subs) > 16):
                        subs.popleft()()
                    g += 1
            emit_PV(*pending)
            while pe_extras:
                pe_extras.popleft()()
            while subs:
                subs.popleft()()
            if KDBG:
                nc.sync.dma_start(dbg["d_vt0"][:], VT[0][:])
                nc.sync.dma_start(dbg["d_vt1"][:], VT[1][:])
                nc.sync.dma_start(dbg["d_qt0"][:], QT[0][:])
                nc.sync.dma_start(dbg["d_kt0"][:], KT[0][:])
                nc.sync.dma_start(dbg["d_va0"][:], VA[0][:])
    nc.compile()
    return nc


def _in_maps(x, w_qkv, w_out):
    import ml_dtypes
    bf16 = ml_dtypes.bfloat16
    x = np.asarray(x, dtype=np.float32)
    w_qkv = np.asarray(w_qkv, dtype=np.float32)
    w_out = np.asarray(w_out, dtype=np.float32)
    aux_const = np.zeros((128, 132), dtype=np.float32)
    aux_const[:, 0:128] = np.eye(128, dtype=np.float32)
    aux_const[:, 128] = 1.0
    aux_const = aux_const.astype(bf16)
    scale = np.float32(1.0 / np.sqrt(DH))
    in_maps = []
    for c in range(NCORES):
        b, g = divmod(c, 4)
        cols = slice(256 * g, 256 * (g + 1))
        in_maps.append({
            "xT": np.ascontiguousarray(x[b].T).astype(bf16),
            "wq": (np.ascontiguousarray(w_qkv[:, 0 * H:1 * H][:, cols])
                   * scale).astype(bf16),
            "wk": np.ascontiguousarray(
                w_qkv[:, 1 * H:2 * H][:, cols]).astype(bf16),
            "wv": np.ascontiguousarray(
                w_qkv[:, 2 * H:3 * H][:, cols]).astype(bf16),
            "wo": np.ascontiguousarray(w_out[cols, :]).astype(bf16),
            "aux": aux_const,
        })
    return in_maps


TRACE = False
LAST_RESULTS = None


def kernel(x, w_qkv, w_out):
    global LAST_RESULTS
    if "nc" not in _CACHE:
        _CACHE["nc"] = _build()
    nc = _CACHE["nc"]
    in_maps = _in_maps(x, w_qkv, w_out)
    res = bass_utils.run_bass_kernel_spmd(
        nc, in_maps, core_ids=list(range(NCORES)), trace=TRACE)
    LAST_RESULTS = res
    y = np.zeros((B, S, H), dtype=np.float32)
    for c in range(NCORES):
        y[c // 4] += np.asarray(res.results[c]["y"], dtype=np.float32)
    return y


# revision 13
# speedup vs baseline: 1.2318x; 1.0301x over previous
"""Causal attention block (B=2, S=2048, H=1024, 16 heads) on 8 NeuronCores.

Sharding: core c handles batch b = c // 4 and head-group g = c % 4
(4 heads = 256 qkv columns / w_out rows per core). Each core computes a
partial output y_partial = softmax(QK^T/sqrt(d)) V @ Wout_slice for its
heads; the host sums the 4 head-group partials per batch.

v4 design (bf16 compute, f32 PSUM):
  x^T [1024, 2048] bf16; Q^T,K^T head-pair tiles [128, 2048] bf16;
  V natural [t, d] per t-chunk (VA [128, 256] bf16).
  S^T per (head, t-chunk, s-chunk 512) -> exp on ACT -> pt bf16; diagonal
  triangle zeroed post-exp by gpsimd affine_select.
  PV flipped to [s, d]: pt block is the stationary operand, VA the 64-wide
  moving operand; softmax denominators via N=1 matmuls against a ones
  column (Z lands per-partition), normalize = per-partition scalar mul,
  then one PE transpose per head pair back to [d, s] for the out-proj.
  Global software pipeline: PV lags S/exp by one step, transposes by two;
  QKV chains for the next chunk are deadline-interleaved as PE filler;
  out-projections are deferred (half-H sub-units) into the ACT-bound tail.
  PSUM: one shared 5-buf tag for scores/chains/out-proj + 2 PV banks +
  1 bank for Z columns and transpose slots.
"""

import numpy as np
from collections import deque
from contextlib import ExitStack

import concourse.bass as bass
import concourse.tile as tile
import concourse.mybir as mybir
from concourse import bacc
from concourse import bass_utils

F32 = mybir.dt.float32
BF16 = mybir.dt.bfloat16
AF = mybir.ActivationFunctionType

B, S, H = 2, 2048, 1024
NH, DH = 16, 64
NCORES = 8
SC = 512            # s-chunk width
NSC = S // SC       # 4
NHC = H // 128      # 8 h contraction chunks

_CACHE = {}


def _gstep(j):
    return 2 * j * (j + 1)


def _build():
    nc = bacc.Bacc("TRN2", target_bir_lowering=False, debug=False,
                   enable_asserts=False, num_devices=NCORES)
    xT = nc.dram_tensor("xT", [H, S], BF16, kind="ExternalInput").ap()
    wq = nc.dram_tensor("wq", [H, 256], BF16, kind="ExternalInput").ap()
    wk = nc.dram_tensor("wk", [H, 256], BF16, kind="ExternalInput").ap()
    wv = nc.dram_tensor("wv", [H, 256], BF16, kind="ExternalInput").ap()
    wo = nc.dram_tensor("wo", [256, H], BF16, kind="ExternalInput").ap()
    aux = nc.dram_tensor("aux", [128, 132], BF16, kind="ExternalInput").ap()
    y = nc.dram_tensor("y", [S, H], BF16, kind="ExternalOutput").ap()
    import os
    KDBG = os.environ.get("KDBG", "0") == "1"
    if KDBG:
        dbg = {nm: nc.dram_tensor(nm, shp, dt, kind="ExternalOutput").ap()
               for nm, shp, dt in [
                   ("d_vt0", [128, S], BF16), ("d_vt1", [128, S], BF16),
                   ("d_qt0", [128, S], BF16), ("d_kt0", [128, S], BF16),
                   ("d_va0", [128, 256], BF16), ("d_zz0", [128, 16], F32),
                   ("d_v2_0", [128, 256], BF16), ("d_pt00", [128, SC], BF16),
                   ("d_pt01", [128, SC], BF16), ("d_pv1", [128, 64], F32),
                   ("d_v2_1", [128, 256], BF16),
               ]}

    with tile.TileContext(nc) as tc:
        with ExitStack() as ctx:
            pw = ctx.enter_context(tc.tile_pool(name="w", bufs=1))
            pxt = ctx.enter_context(tc.tile_pool(name="xt", bufs=2))
            pbig = ctx.enter_context(tc.tile_pool(name="big", bufs=1))
            ppt = ctx.enter_context(tc.tile_pool(name="pt", bufs=36))
            pzz = ctx.enter_context(tc.tile_pool(name="zz", bufs=2))
            pv2 = ctx.enter_context(tc.tile_pool(name="v2", bufs=3))
            pyo = ctx.enter_context(tc.tile_pool(name="yo", bufs=4))
            psum = ctx.enter_context(
                tc.tile_pool(name="psum", bufs=1, space="PSUM"))

            def s_tile(name):
                # 2-bank slots shared by score-pairs, chains, out-proj, tr
                return psum.tile([128, 2 * SC], F32, tag="s", bufs=3,
                                 name=name)

            # ---- weights on the scalar (ACT) DGE queue, before any exp ----
            def load_w_all(dram, nm, nsplit=1):
                t = pw.tile([128, NHC * 256], BF16, tag=nm, name=nm)
                dst = t[:].rearrange("p (c n) -> p c n", c=NHC)
                src = dram.rearrange("(c p) n -> p c n", p=128)
                step = NHC // nsplit
                for si in range(nsplit):
                    nc.scalar.dma_start(
                        dst[:, si * step:(si + 1) * step, :],
                        src[:, si * step:(si + 1) * step, :])
                return [t[:, hc * 256:(hc + 1) * 256] for hc in range(NHC)]

            wq_t = load_w_all(wq, "wqa", nsplit=4)
            wk_t = load_w_all(wk, "wka")
            aux_t = pw.tile([128, 132], BF16, tag="aux")
            nc.scalar.dma_start(aux_t[:], aux[:])
            ident = aux_t[:, 0:128]
            ones1 = aux_t[:, 128:129]
            wv_t = load_w_all(wv, "wva")
            wo_t = []
            for p in range(2):
                t = pw.tile([128, H], BF16, tag=f"wo{p}", name=f"wo{p}")
                nc.scalar.dma_start(t[:], wo[p * 128:(p + 1) * 128, :])
                wo_t.append(t)

            # ---- persistent activations ----
            QT = [pbig.tile([128, S], BF16, tag=f"qt{p}", name=f"qt{p}")
                  for p in range(2)]
            KT = [pbig.tile([128, S], BF16, tag=f"kt{p}", name=f"kt{p}")
                  for p in range(2)]
            VT = [pbig.tile([128, S], BF16, tag=f"vt{p}", name=f"vt{p}")
                  for p in range(2)]
            VA = [pbig.tile([128, 260], BF16, tag=f"va{t_}", name=f"va{t_}")
                  for t_ in range(S // 128)]
            for t_ in range(S // 128):
                ones_cols = VA[t_][:].rearrange(
                    "q (h c) -> q h c", c=65)[:, :, 64]
                nc.gpsimd.memset(ones_cols, 1.0)

            xt_tiles = [None] * NSC

            def load_xt(j, nsplit):
                xt_all = pxt.tile([128, NHC * SC], BF16, tag="xt",
                                  name=f"xt{j}")
                sj = slice(j * SC, (j + 1) * SC)
                xt_src = xT.rearrange("(c p) s -> p c s", p=128)[:, :, sj]
                xt_dst = xt_all[:].rearrange("p (c s) -> p c s", c=NHC)
                step = NHC // nsplit
                for si in range(nsplit):
                    nc.sync.dma_start(
                        xt_dst[:, si * step:(si + 1) * step, :],
                        xt_src[:, si * step:(si + 1) * step, :])
                xt_tiles[j] = xt_all

            # ---- QKV projection chains ----
            def qk_chain(j, W, OUT, p):
                def emit():
                    xt_all = xt_tiles[j]
                    sj = slice(j * SC, (j + 1) * SC)
                    ps = psum.tile([128, SC], F32, tag="pv", bufs=2,
                                   name=f"qk{j}_{p}")
                    for hc in range(NHC):
                        nc.tensor.matmul(
                            ps[:], W[hc][:, p * 128:(p + 1) * 128],
                            xt_all[:, hc * SC:(hc + 1) * SC],
                            start=(hc == 0), stop=(hc == NHC - 1))
                    nc.vector.tensor_copy(OUT[p][:, sj], ps[:])
                return emit

            def v_chain(j, tci):
                def emit():
                    xt_all = xt_tiles[j]
                    t_ = 4 * j + tci
                    ps = psum.tile([128, SC], F32, tag="pv", bufs=2,
                                   name=f"v{j}_{tci}")
                    for hc in range(NHC):
                        nc.tensor.matmul(
                            ps[:, 0:256],
                            xt_all[:, hc * SC + tci * 128:
                                   hc * SC + (tci + 1) * 128],
                            wv_t[hc], start=(hc == 0), stop=(hc == NHC - 1))
                    dst = VA[t_][:].rearrange(
                        "q (h c) -> q h c", c=65)[:, :, 0:64]
                    nc.vector.tensor_copy(
                        dst, ps[:, 0:256].rearrange(
                            "q (h c) -> q h c", c=64))
                return emit

            # ---- chunk-local state: pt tiles persist per chunk ----
            class ChunkCtx:
                def __init__(self, j):
                    self.pts = {}   # (tcc, h) -> pt tile
                    self.zz = pzz.tile([128, 16], F32, tag="zz",
                                       name=f"zz{j}")

            # ---- attention pieces ----
            def emit_S(cc, j, tcc, prs):
                k = tcc - 4 * j
                c0 = max(0, 128 * k)
                sjv = slice(j * SC + c0, (j + 1) * SC)
                for p in prs:
                    ss = s_tile(f"ss{tcc}_{p}")
                    for r in range(2):
                        nc.tensor.matmul(
                            ss[:, SC * r + c0:SC * (r + 1)],
                            KT[p][64 * r:64 * (r + 1),
                                  tcc * 128:(tcc + 1) * 128],
                            QT[p][64 * r:64 * (r + 1), sjv],
                            start=True, stop=True)
                    pt = ppt.tile([128, 2 * SC], BF16, tag="pt")
                    w2 = SC - c0
                    src2 = ss[:].rearrange("q (r s) -> q r s", r=2)[
                        :, :, c0:SC]
                    dst2 = pt[:].rearrange("q (r s) -> q r s", r=2)[
                        :, :, c0:SC]
                    nc.scalar.activation(dst2, src2, AF.Exp)
                    if k >= 0:
                        band = pt[:].rearrange("q (r s) -> q r s", r=2)[
                            :, :, c0:c0 + 128]
                        nc.gpsimd.affine_select(
                            band, band,
                            pattern=[[0, 2], [1, 128]], base=0,
                            channel_multiplier=-1,
                            compare_op=mybir.AluOpType.is_ge, fill=0.0)
                    cc.pts[(tcc, p)] = pt

            subs = deque()        # deferred out-projection sub-closures
            pe_extras = deque()   # deferred transpose closures

            def make_transpose(cc, j, sti, v2):
                st = 4 * j + sti

                def emit():
                    for p in range(2):
                        trt = s_tile(f"tr{st}_{p}")[:, 0:SC]
                        trp = trt[:, 0:64].bitcast(BF16)
                        nc.tensor.transpose(
                            trp, v2[:, 128 * p:128 * (p + 1)], ident)
                        nc.vector.tensor_copy(
                            VT[p][:, st * 128:(st + 1) * 128], trp)
                    ysb = pyo.tile([128, H], BF16, tag="y", name=f"ysb{st}")
                    for n2 in range(2):
                        subs.append(make_sub(st, n2, ysb))
                return emit

            def make_sub(st, n2, ysb):
                def emit():
                    py_ = psum.tile([128, SC], F32, tag="pv", bufs=2,
                                    name=f"py{st}_{n2}")
                    for p in range(2):
                        nc.tensor.matmul(
                            py_[:], VT[p][:, st * 128:(st + 1) * 128],
                            wo_t[p][:, n2 * 512:(n2 + 1) * 512],
                            start=(p == 0), stop=(p == 1))
                    nc.vector.tensor_copy(
                        ysb[:, n2 * 512:(n2 + 1) * 512], py_[:])
                    nc.sync.dma_start(
                        y[st * 128:(st + 1) * 128, n2 * 512:(n2 + 1) * 512],
                        ysb[:, n2 * 512:(n2 + 1) * 512])
                return emit

            def emit_PV(cc, j, sti):
                # one unbroken accumulation group per (s-tile, head):
                # PV over tcc=0..st, then z over tcc=0..st, sequentially
                # through one psum bank (one open group per bank at a time)
                st = 4 * j + sti
                bank = psum.tile([128, SC], F32, tag="pv", bufs=2,
                                 name=f"pv{st}")
                for h in range(4):
                    p_, r_ = divmod(h, 2)
                    o_ = SC * r_ + sti * 128
                    for tcc in range(st + 1):
                        ptsl = cc.pts[(tcc, p_)][:, o_:o_ + 128]
                        nc.tensor.matmul(
                            bank[:, 65 * h:65 * (h + 1)], ptsl,
                            VA[tcc][:, 65 * h:65 * (h + 1)],
                            start=(tcc == 0), stop=(tcc == st))
                nc.vector.reciprocal(
                    cc.zz[:, 4 * sti:4 * sti + 4],
                    bank[:, 0:260].rearrange(
                        "q (h c) -> q h c", c=65)[:, :, 64])
                v2 = pv2.tile([128, 256], BF16, tag="v2", name=f"v2_{st}")
                for h in range(4):
                    nc.vector.tensor_scalar_mul(
                        v2[:, 64 * h:64 * (h + 1)],
                        bank[:, 65 * h:65 * h + 64],
                        cc.zz[:, 4 * sti + h:4 * sti + h + 1])
                if KDBG and st == 0:
                    nc.sync.dma_start(dbg["d_v2_0"][:], v2[:])
                if KDBG and st == 1:
                    nc.sync.dma_start(dbg["d_zz0"][:], cc.zz[:])
                    nc.sync.dma_start(dbg["d_v2_1"][:], v2[:])
                    dsb = pw.tile([128, 64], F32, tag="dsb")
                    nc.vector.tensor_copy(dsb[:], bank[:, 0:64])
                    nc.sync.dma_start(dbg["d_pv1"][:], dsb[:])
                pe_extras.append(make_transpose(cc, j, sti, v2))

            # ---- global schedule ----
            chains = deque()   # (deadline_step, emit_fn)

            load_xt(0, 8)
            qk_chain(0, wq_t, QT, 0)()
            qk_chain(0, wk_t, KT, 0)()
            qk_chain(0, wq_t, QT, 1)()
            qk_chain(0, wk_t, KT, 1)()
            for tci in range(4):
                chains.append((tci + 1, v_chain(0, tci)))

            pending = None
            g = 0
            for j in range(NSC):
                ntc = 4 * j + 4
                cc = ChunkCtx(j)
                if j + 1 < NSC:
                    load_xt(j + 1, 2)
                    g1 = _gstep(j + 1)
                    for p in range(2):
                        chains.append((g1, qk_chain(j + 1, wq_t, QT, p)))
                    for p in range(2):
                        chains.append((g1 + 4 * (j + 1),
                                       qk_chain(j + 1, wk_t, KT, p)))
                    for tci in range(4):
                        chains.append((g1 + 4 * (j + 1) + tci,
                                       v_chain(j + 1, tci)))
                for tcc in range(ntc):
                    k = tcc - 4 * j
                    emit_S(cc, j, tcc, [0])
                    while pe_extras:
                        pe_extras.popleft()()
                    if pending is not None:
                        emit_PV(*pending)
                        pending = None
                    emit_S(cc, j, tcc, [1])
                    if k >= 0:
                        pending = (cc, j, k)
                    # non-S psum users this step: <= 1 chain + <= 1 sub
                    popped = 0
                    while chains and (chains[0][0] <= g + 1 or popped == 0):
                        chains.popleft()[1]()
                        popped += 1
                        if popped >= 2 and not (chains
                                                and chains[0][0] <= g + 1):
                            break
                    nsub = 0 if k >= 0 else (2 if j >= 2 else 1)
                    if len(subs) <= 8 and j < 3:
                        nsub = 0
                    for _ in range(min(nsub, len(subs))):
                        subs.popleft()()
                    g += 1
            emit_PV(*pending)
            while pe_extras:
                pe_extras.popleft()()
            while subs:
                subs.popleft()()
            if KDBG:
                nc.sync.dma_start(dbg["d_vt0"][:], VT[0][:])
                nc.sync.dma_start(dbg["d_vt1"][:], VT[1][:])
                nc.sync.dma_start(dbg["d_qt0"][:], QT[0][:])
                nc.sync.dma_start(dbg["d_kt0"][:], KT[0][:])
                nc.sync.dma_start(dbg["d_va0"][:], VA[0][:])
    nc.compile()
    return nc


def _in_maps(x, w_qkv, w_out):
    import ml_dtypes
    bf16 = ml_dtypes.bfloat16
    x = np.asarray(x, dtype=np.float32)
    w_qkv = np.asarray(w_qkv, dtype=np.float32)
    w_out = np.asarray(w_out, dtype=np.float32)
    aux_const = np.zeros((128, 132), dtype=np.float32)
    aux_const[:, 0:128] = np.eye(128, dtype=np.float32)
    aux_const[:, 128] = 1.0
    aux_const = aux_const.astype(bf16)
    scale = np.float32(1.0 / np.sqrt(DH))
    in_maps = []
    for c in range(NCORES):
        b, g = divmod(c, 4)
        cols = slice(256 * g, 256 * (g + 1))
        in_maps.append({
            "xT": np.ascontiguousarray(x[b].T).astype(bf16),
            "wq": (np.ascontiguousarray(w_qkv[:, 0 * H:1 * H][:, cols])
                   * scale).astype(bf16),
            "wk": np.ascontiguousarray(
                w_qkv[:, 1 * H:2 * H][:, cols]).astype(bf16),
            "wv": np.ascontiguousarray(
                w_qkv[:, 2 * H:3 * H][:, cols]).astype(bf16),
            "wo": np.ascontiguousarray(w_out[cols, :]).astype(bf16),
            "aux": aux_const,
        })
    return in_maps


TRACE = False
LAST_RESULTS = None


def kernel(x, w_qkv, w_out):
    global LAST_RESULTS
    if "nc" not in _CACHE:
        _CACHE["nc"] = _build()
    nc = _CACHE["nc"]
    in_maps = _in_maps(x, w_qkv, w_out)
    res = bass_utils.run_bass_kernel_spmd(
        nc, in_maps, core_ids=list(range(NCORES)), trace=TRACE)
    LAST_RESULTS = res
    y = np.zeros((B, S, H), dtype=np.float32)
    for c in range(NCORES):
        y[c // 4] += np.asarray(res.results[c]["y"], dtype=np.float32)
    return y


# revision 23
# speedup vs baseline: 1.3236x; 1.0746x over previous
"""Causal attention block (B=2, S=2048, H=1024, 16 heads) on 8 NeuronCores.

Sharding: core c handles batch b = c // 4 and head-group g = c % 4
(4 heads = 256 qkv columns / w_out rows per core). Each core computes a
partial output y_partial = softmax(QK^T/sqrt(d)) V @ Wout_slice for its
heads; the host sums the 4 head-group partials per batch.

v4 design (bf16 compute, f32 PSUM):
  x^T [1024, 2048] bf16; Q^T,K^T head-pair tiles [128, 2048] bf16;
  V natural [t, d] per t-chunk (VA [128, 256] bf16).
  S^T per (head, t-chunk, s-chunk 512) -> exp on ACT -> pt bf16; diagonal
  triangle zeroed post-exp by gpsimd affine_select.
  PV flipped to [s, d]: pt block is the stationary operand, VA the 64-wide
  moving operand; softmax denominators via N=1 matmuls against a ones
  column (Z lands per-partition), normalize = per-partition scalar mul,
  then one PE transpose per head pair back to [d, s] for the out-proj.
  Global software pipeline: PV lags S/exp by one step, transposes by two;
  QKV chains for the next chunk are deadline-interleaved as PE filler;
  out-projections are deferred (half-H sub-units) into the ACT-bound tail.
  PSUM: one shared 5-buf tag for scores/chains/out-proj + 2 PV banks +
  1 bank for Z columns and transpose slots.
"""

import numpy as np
from collections import deque
from contextlib import ExitStack

import concourse.bass as bass
import concourse.tile as tile
import concourse.mybir as mybir
from concourse import bacc
from concourse import bass_utils

F32 = mybir.dt.float32
BF16 = mybir.dt.bfloat16
AF = mybir.ActivationFunctionType

B, S, H = 2, 2048, 1024
NH, DH = 16, 64
NCORES = 8
SC = 512            # s-chunk width
NSC = S // SC       # 4
NHC = H // 128      # 8 h contraction chunks

_CACHE = {}


def _gstep(j):
    return 2 * j * (j + 1)


def _build():
    nc = bacc.Bacc("TRN2", target_bir_lowering=False, debug=False,
                   enable_asserts=False, num_devices=NCORES)
    xT = nc.dram_tensor("xT", [H, S], BF16, kind="ExternalInput").ap()
    wq = nc.dram_tensor("wq", [H, 256], BF16, kind="ExternalInput").ap()
    wk = nc.dram_tensor("wk", [H, 256], BF16, kind="ExternalInput").ap()
    wv = nc.dram_tensor("wv", [H, 256], BF16, kind="ExternalInput").ap()
    wo = nc.dram_tensor("wo", [256, H], BF16, kind="ExternalInput").ap()
    aux = nc.dram_tensor("aux", [128, 132], BF16, kind="ExternalInput").ap()
    y = nc.dram_tensor("y", [S, H], BF16, kind="ExternalOutput").ap()
    import os
    KDBG = os.environ.get("KDBG", "0") == "1"
    if KDBG:
        dbg = {nm: nc.dram_tensor(nm, shp, dt, kind="ExternalOutput").ap()
               for nm, shp, dt in [
                   ("d_vt0", [128, S], BF16), ("d_vt1", [128, S], BF16),
                   ("d_qt0", [128, S], BF16), ("d_kt0", [128, S], BF16),
                   ("d_va0", [128, 256], BF16), ("d_zz0", [128, 16], F32),
                   ("d_v2_0", [128, 256], BF16), ("d_pt00", [128, SC], BF16),
                   ("d_pt01", [128, SC], BF16), ("d_pv1", [128, 64], F32),
                   ("d_v2_1", [128, 256], BF16),
               ]}

    with tile.TileContext(nc) as tc:
        with ExitStack() as ctx:
            pw = ctx.enter_context(tc.tile_pool(name="w", bufs=1))
            pxt = ctx.enter_context(tc.tile_pool(name="xt", bufs=2))
            pbig = ctx.enter_context(tc.tile_pool(name="big", bufs=1))
            ppt = ctx.enter_context(tc.tile_pool(name="pt", bufs=36))
            pzz = ctx.enter_context(tc.tile_pool(name="zz", bufs=2))
            pv2 = ctx.enter_context(tc.tile_pool(name="v2", bufs=3))
            pyo = ctx.enter_context(tc.tile_pool(name="yo", bufs=4))
            psum = ctx.enter_context(
                tc.tile_pool(name="psum", bufs=1, space="PSUM"))

            def s_tile(name):
                # 2-bank slots shared by score-pairs, chains, out-proj, tr
                return psum.tile([128, 2 * SC], F32, tag="s", bufs=3,
                                 name=name)

            # ---- weights on the scalar (ACT) DGE queue, before any exp ----
            def load_w_all(dram, nm, nsplit=1):
                t = pw.tile([128, NHC * 256], BF16, tag=nm, name=nm)
                dst = t[:].rearrange("p (c n) -> p c n", c=NHC)
                src = dram.rearrange("(c p) n -> p c n", p=128)
                step = NHC // nsplit
                for si in range(nsplit):
                    nc.scalar.dma_start(
                        dst[:, si * step:(si + 1) * step, :],
                        src[:, si * step:(si + 1) * step, :])
                return [t[:, hc * 256:(hc + 1) * 256] for hc in range(NHC)]

            wq_t = load_w_all(wq, "wqa", nsplit=2)
            wk_t = load_w_all(wk, "wka")
            aux_t = pw.tile([128, 132], BF16, tag="aux")
            nc.scalar.dma_start(aux_t[:], aux[:])
            ident = aux_t[:, 0:128]
            ones1 = aux_t[:, 128:129]
            wv_t = load_w_all(wv, "wva")
            wo_t = []
            for p in range(2):
                t = pw.tile([128, H], BF16, tag=f"wo{p}", name=f"wo{p}")
                nc.scalar.dma_start(t[:], wo[p * 128:(p + 1) * 128, :])
                wo_t.append(t)

            # ---- persistent activations ----
            QT = [pbig.tile([128, S], BF16, tag=f"qt{p}", name=f"qt{p}")
                  for p in range(2)]
            KT = [pbig.tile([128, S], BF16, tag=f"kt{p}", name=f"kt{p}")
                  for p in range(2)]
            VT = [pbig.tile([128, S], BF16, tag=f"vt{p}", name=f"vt{p}")
                  for p in range(2)]
            VA = [pbig.tile([128, 260], BF16, tag=f"va{t_}", name=f"va{t_}")
                  for t_ in range(S // 128)]
            for t_ in range(S // 128):
                ones_cols = VA[t_][:].rearrange(
                    "q (h c) -> q h c", c=65)[:, :, 64]
                nc.gpsimd.memset(ones_cols, 1.0)

            xt_tiles = [None] * NSC

            def load_xt(j, nsplit):
                xt_all = pxt.tile([128, NHC * SC], BF16, tag="xt",
                                  name=f"xt{j}")
                sj = slice(j * SC, (j + 1) * SC)
                xt_src = xT.rearrange("(c p) s -> p c s", p=128)[:, :, sj]
                xt_dst = xt_all[:].rearrange("p (c s) -> p c s", c=NHC)
                step = NHC // nsplit
                for si in range(nsplit):
                    nc.sync.dma_start(
                        xt_dst[:, si * step:(si + 1) * step, :],
                        xt_src[:, si * step:(si + 1) * step, :])
                xt_tiles[j] = xt_all

            # ---- QKV projection chains ----
            def qk_chain(j, W, OUT, p, tag="pv"):
                def emit():
                    xt_all = xt_tiles[j]
                    sj = slice(j * SC, (j + 1) * SC)
                    if tag == "s":
                        ps = s_tile(f"qk{j}_{p}")[:, 0:SC]
                    else:
                        ps = psum.tile([128, SC], F32, tag="pv", bufs=2,
                                       name=f"qk{j}_{p}")
                    for hc in range(NHC):
                        nc.tensor.matmul(
                            ps[:], W[hc][:, p * 128:(p + 1) * 128],
                            xt_all[:, hc * SC:(hc + 1) * SC],
                            start=(hc == 0), stop=(hc == NHC - 1))
                    nc.vector.tensor_copy(OUT[p][:, sj], ps[:])
                return emit

            def v_chain(j, tci):
                def emit():
                    xt_all = xt_tiles[j]
                    t_ = 4 * j + tci
                    ps = psum.tile([128, SC], F32, tag="pv", bufs=2,
                                   name=f"v{j}_{tci}")
                    for hc in range(NHC):
                        nc.tensor.matmul(
                            ps[:, 0:256],
                            xt_all[:, hc * SC + tci * 128:
                                   hc * SC + (tci + 1) * 128],
                            wv_t[hc], start=(hc == 0), stop=(hc == NHC - 1))
                    dst = VA[t_][:].rearrange(
                        "q (h c) -> q h c", c=65)[:, :, 0:64]
                    nc.vector.tensor_copy(
                        dst, ps[:, 0:256].rearrange(
                            "q (h c) -> q h c", c=64))
                return emit

            # ---- chunk-local state: pt tiles persist per chunk ----
            class ChunkCtx:
                def __init__(self, j):
                    self.pts = {}   # (tcc, h) -> pt tile
                    self.zz = pzz.tile([128, 16], F32, tag="zz",
                                       name=f"zz{j}")

            # ---- attention pieces ----
            def emit_S(cc, j, tcc, prs):
                k = tcc - 4 * j
                c0 = max(0, 128 * k)
                sjv = slice(j * SC + c0, (j + 1) * SC)
                for p in prs:
                    ss = s_tile(f"ss{tcc}_{p}")
                    for r in range(2):
                        nc.tensor.matmul(
                            ss[:, SC * r + c0:SC * (r + 1)],
                            KT[p][64 * r:64 * (r + 1),
                                  tcc * 128:(tcc + 1) * 128],
                            QT[p][64 * r:64 * (r + 1), sjv],
                            start=True, stop=True)
                    pt = ppt.tile([128, 2 * SC], BF16, tag="pt")
                    w2 = SC - c0
                    src2 = ss[:].rearrange("q (r s) -> q r s", r=2)[
                        :, :, c0:SC]
                    dst2 = pt[:].rearrange("q (r s) -> q r s", r=2)[
                        :, :, c0:SC]
                    nc.scalar.activation(dst2, src2, AF.Exp)
                    if k >= 0:
                        band = pt[:].rearrange("q (r s) -> q r s", r=2)[
                            :, :, c0:c0 + 128]
                        nc.gpsimd.affine_select(
                            band, band,
                            pattern=[[0, 2], [1, 128]], base=0,
                            channel_multiplier=-1,
                            compare_op=mybir.AluOpType.is_ge, fill=0.0)
                    cc.pts[(tcc, p)] = pt

            subs = deque()        # deferred out-projection sub-closures
            pe_extras = deque()   # deferred transpose closures

            def make_transpose(cc, j, sti, v2):
                st = 4 * j + sti

                def emit():
                    for p in range(2):
                        trt = s_tile(f"tr{st}_{p}")[:, 0:SC]
                        trp = trt[:, 0:64].bitcast(BF16)
                        nc.tensor.transpose(
                            trp, v2[:, 128 * p:128 * (p + 1)], ident)
                        nc.vector.tensor_copy(
                            VT[p][:, st * 128:(st + 1) * 128], trp)
                    ysb = pyo.tile([128, H], BF16, tag="y", name=f"ysb{st}")
                    for n2 in range(2):
                        subs.append(make_sub(st, n2, ysb))
                return emit

            def make_sub(st, n2, ysb):
                def emit():
                    py_ = psum.tile([128, SC], F32, tag="pv", bufs=2,
                                    name=f"py{st}_{n2}")
                    for p in range(2):
                        nc.tensor.matmul(
                            py_[:], VT[p][:, st * 128:(st + 1) * 128],
                            wo_t[p][:, n2 * 512:(n2 + 1) * 512],
                            start=(p == 0), stop=(p == 1))
                    if st >= 12 and n2 == 1:
                        nc.scalar.copy(
                            ysb[:, n2 * 512:(n2 + 1) * 512], py_[:])
                    else:
                        nc.vector.tensor_copy(
                            ysb[:, n2 * 512:(n2 + 1) * 512], py_[:])
                    nc.sync.dma_start(
                        y[st * 128:(st + 1) * 128, n2 * 512:(n2 + 1) * 512],
                        ysb[:, n2 * 512:(n2 + 1) * 512])
                return emit

            def emit_PV(cc, j, sti):
                # one unbroken accumulation group per (s-tile, head):
                # PV over tcc=0..st, then z over tcc=0..st, sequentially
                # through one psum bank (one open group per bank at a time)
                st = 4 * j + sti
                bank = psum.tile([128, SC], F32, tag="pv", bufs=2,
                                 name=f"pv{st}")
                for h in range(4):
                    p_, r_ = divmod(h, 2)
                    o_ = SC * r_ + sti * 128
                    for tcc in range(st + 1):
                        ptsl = cc.pts[(tcc, p_)][:, o_:o_ + 128]
                        nc.tensor.matmul(
                            bank[:, 65 * h:65 * (h + 1)], ptsl,
                            VA[tcc][:, 65 * h:65 * (h + 1)],
                            start=(tcc == 0), stop=(tcc == st))
                nc.vector.reciprocal(
                    cc.zz[:, 4 * sti:4 * sti + 4],
                    bank[:, 0:260].rearrange(
                        "q (h c) -> q h c", c=65)[:, :, 64])
                v2 = pv2.tile([128, 256], BF16, tag="v2", name=f"v2_{st}")
                for h in range(4):
                    nc.vector.tensor_scalar_mul(
                        v2[:, 64 * h:64 * (h + 1)],
                        bank[:, 65 * h:65 * h + 64],
                        cc.zz[:, 4 * sti + h:4 * sti + h + 1])
                if KDBG and st == 0:
                    nc.sync.dma_start(dbg["d_v2_0"][:], v2[:])
                if KDBG and st == 1:
                    nc.sync.dma_start(dbg["d_zz0"][:], cc.zz[:])
                    nc.sync.dma_start(dbg["d_v2_1"][:], v2[:])
                    dsb = pw.tile([128, 64], F32, tag="dsb")
                    nc.vector.tensor_copy(dsb[:], bank[:, 0:64])
                    nc.sync.dma_start(dbg["d_pv1"][:], dsb[:])
                pe_extras.append(make_transpose(cc, j, sti, v2))

            # ---- global schedule ----
            chains = deque()   # (deadline_step, emit_fn)

            load_xt(0, 4)
            qk_chain(0, wq_t, QT, 0, tag="pv")()
            qk_chain(0, wk_t, KT, 0, tag="s")()
            qk_chain(0, wq_t, QT, 1, tag="s")()
            qk_chain(0, wk_t, KT, 1, tag="s")()
            for tci in range(4):
                chains.append((tci + 1, v_chain(0, tci)))

            pending = None
            g = 0
            for j in range(NSC):
                ntc = 4 * j + 4
                cc = ChunkCtx(j)
                if j + 1 < NSC:
                    load_xt(j + 1, 2)
                    g1 = _gstep(j + 1)
                    for p in range(2):
                        chains.append((g1, qk_chain(j + 1, wq_t, QT, p)))
                    for p in range(2):
                        chains.append((g1 + 4 * (j + 1),
                                       qk_chain(j + 1, wk_t, KT, p)))
                    for tci in range(4):
                        chains.append((g1 + 4 * (j + 1) + tci,
                                       v_chain(j + 1, tci)))
                for tcc in range(ntc):
                    k = tcc - 4 * j
                    emit_S(cc, j, tcc, [0])
                    while pe_extras:
                        pe_extras.popleft()()
                    if pending is not None:
                        emit_PV(*pending)
                        pending = None
                    emit_S(cc, j, tcc, [1])
                    if k >= 0:
                        pending = (cc, j, k)
                    # non-S psum users this step: <= 1 chain + <= 1 sub
                    popped = 0
                    while chains and (chains[0][0] <= g + 1
                                      or (popped == 0
                                          and chains[0][0] <= g + 6)):
                        chains.popleft()[1]()
                        popped += 1
                        if popped >= 2 and not (chains
                                                and chains[0][0] <= g + 1):
                            break
                    nsub = 2 if (j == 3 and k < 0) else 0
                    for _ in range(min(nsub, len(subs))):
                        subs.popleft()()
                    g += 1
            emit_PV(*pending)
            while pe_extras:
                pe_extras.popleft()()
            while subs:
                subs.popleft()()
            if KDBG:
                nc.sync.dma_start(dbg["d_vt0"][:], VT[0][:])
                nc.sync.dma_start(dbg["d_vt1"][:], VT[1][:])
                nc.sync.dma_start(dbg["d_qt0"][:], QT[0][:])
                nc.sync.dma_start(dbg["d_kt0"][:], KT[0][:])
                nc.sync.dma_start(dbg["d_va0"][:], VA[0][:])
    nc.compile()
    return nc


def _in_maps(x, w_qkv, w_out):
    import ml_dtypes
    bf16 = ml_dtypes.bfloat16
    x = np.asarray(x, dtype=np.float32)
    w_qkv = np.asarray(w_qkv, dtype=np.float32)
    w_out = np.asarray(w_out, dtype=np.float32)
    aux_const = np.zeros((128, 132), dtype=np.float32)
    aux_const[:, 0:128] = np.eye(128, dtype=np.float32)
    aux_const[:, 128] = 1.0
    aux_const = aux_const.astype(bf16)
    scale = np.float32(1.0 / np.sqrt(DH))
    in_maps = []
    for c in range(NCORES):
        b, g = divmod(c, 4)
        cols = slice(256 * g, 256 * (g + 1))
        in_maps.append({
            "xT": np.ascontiguousarray(x[b].T).astype(bf16),
            "wq": (np.ascontiguousarray(w_qkv[:, 0 * H:1 * H][:, cols])
                   * scale).astype(bf16),
            "wk": np.ascontiguousarray(
                w_qkv[:, 1 * H:2 * H][:, cols]).astype(bf16),
            "wv": np.ascontiguousarray(
                w_qkv[:, 2 * H:3 * H][:, cols]).astype(bf16),
            "wo": np.ascontiguousarray(w_out[cols, :]).astype(bf16),
            "aux": aux_const,
        })
    return in_maps


TRACE = False
LAST_RESULTS = None


def kernel(x, w_qkv, w_out):
    global LAST_RESULTS
    if "nc" not in _CACHE:
        _CACHE["nc"] = _build()
    nc = _CACHE["nc"]
    in_maps = _in_maps(x, w_qkv, w_out)
    res = bass_utils.run_bass_kernel_spmd(
        nc, in_maps, core_ids=list(range(NCORES)), trace=TRACE)
    LAST_RESULTS = res
    y = np.zeros((B, S, H), dtype=np.float32)
    for c in range(NCORES):
        y[c // 4] += np.asarray(res.results[c]["y"], dtype=np.float32)
    return y


# revision 38
# speedup vs baseline: 1.3259x; 1.0017x over previous
"""Causal attention block (B=2, S=2048, H=1024, 16 heads) on 8 NeuronCores.

Sharding: core c handles batch b = c // 4 and head-group g = c % 4
(4 heads = 256 qkv columns / w_out rows per core). Each core computes a
partial output y_partial = softmax(QK^T/sqrt(d)) V @ Wout_slice for its
heads; the host sums the 4 head-group partials per batch.

Design (bf16 compute, f32 PSUM):
  x^T [1024, 2048] bf16 per 512-col s-chunk; Q^T,K^T head-pair tiles
  [128, 2048] bf16 (d on partitions); V natural [t, d] per t-chunk in
  VA [128, 260] bf16 with an interleaved ones column per head (cols
  65h+64), so the PV matmul emits softmax denominators for free.
  Scores: S^T = K^T.T @ Q^T per (head-pair, t-chunk, s-chunk) into a
  2-bank psum pair tile; ONE exp per pair (2-region AP) -> persistent
  pt [128, 1024] bf16; diagonal triangles zeroed post-exp by a single
  gpsimd affine_select over both heads (pattern [[0,2],[1,128]]).
  PV flipped to [s, d] orientation: the pt block is the *stationary*
  operand and VA the 65-wide moving operand, so each PV step costs 65
  rows instead of 512. PSUM allows only ONE open accumulation group per
  bank, so each (s-tile, head) accumulates over its whole t-range as one
  unbroken matmul group, batched at diagonal steps (pt tiles persist per
  chunk). Z lands per-partition -> reciprocal + per-partition scalar
  multiply (no broadcast dance), then one PE transpose (identity rhs)
  per head pair into VT [d, s] for the out-projection. Transposes write
  into fresh 2-bank slots only: a transpose clobbers its whole psum bank.
  Schedule: software pipeline with PV batches lagging S/exp one step and
  transposes two; next-chunk QKV chains are deadline-scheduled PE filler;
  all out-projections (half-H sub-units) are deferred into the ACT-bound
  final chunk, with tail ysb copies alternating DVE/ACT.
  PSUM: 3 x 2-bank score slots + 2 rotating banks (PV groups / chains /
  out-proj / transposes).  Queues: x^T + y on sync(SP), weights on
  scalar(ACT) issued before any exp.
"""

import numpy as np
from collections import deque
from contextlib import ExitStack

import concourse.bass as bass
import concourse.tile as tile
import concourse.mybir as mybir
from concourse import bacc
from concourse import bass_utils

F32 = mybir.dt.float32
BF16 = mybir.dt.bfloat16
AF = mybir.ActivationFunctionType

B, S, H = 2, 2048, 1024
NH, DH = 16, 64
NCORES = 8
SC = 512            # s-chunk width
NSC = S // SC       # 4
NHC = H // 128      # 8 h contraction chunks

_CACHE = {}


def _gstep(j):
    return 2 * j * (j + 1)


def _build():
    nc = bacc.Bacc("TRN2", target_bir_lowering=False, debug=False,
                   enable_asserts=False, num_devices=NCORES)
    xT = nc.dram_tensor("xT", [H, S], BF16, kind="ExternalInput").ap()
    wq = nc.dram_tensor("wq", [H, 256], BF16, kind="ExternalInput").ap()
    wk = nc.dram_tensor("wk", [H, 256], BF16, kind="ExternalInput").ap()
    wv = nc.dram_tensor("wv", [H, 256], BF16, kind="ExternalInput").ap()
    wo = nc.dram_tensor("wo", [256, H], BF16, kind="ExternalInput").ap()
    aux = nc.dram_tensor("aux", [128, 132], BF16, kind="ExternalInput").ap()
    y = nc.dram_tensor("y", [S, H], BF16, kind="ExternalOutput").ap()
    import os
    KDBG = os.environ.get("KDBG", "0") == "1"
    if KDBG:
        dbg = {nm: nc.dram_tensor(nm, shp, dt, kind="ExternalOutput").ap()
               for nm, shp, dt in [
                   ("d_vt0", [128, S], BF16), ("d_vt1", [128, S], BF16),
                   ("d_qt0", [128, S], BF16), ("d_kt0", [128, S], BF16),
                   ("d_va0", [128, 256], BF16), ("d_zz0", [128, 16], F32),
                   ("d_v2_0", [128, 256], BF16), ("d_pt00", [128, SC], BF16),
                   ("d_pt01", [128, SC], BF16), ("d_pv1", [128, 64], F32),
                   ("d_v2_1", [128, 256], BF16),
               ]}

    with tile.TileContext(nc) as tc:
        with ExitStack() as ctx:
            pw = ctx.enter_context(tc.tile_pool(name="w", bufs=1))
            pxt = ctx.enter_context(tc.tile_pool(name="xt", bufs=2))
            pbig = ctx.enter_context(tc.tile_pool(name="big", bufs=1))
            ppt = ctx.enter_context(tc.tile_pool(name="pt", bufs=36))
            pzz = ctx.enter_context(tc.tile_pool(name="zz", bufs=2))
            pv2 = ctx.enter_context(tc.tile_pool(name="v2", bufs=3))
            pyo = ctx.enter_context(tc.tile_pool(name="yo", bufs=4))
            psum = ctx.enter_context(
                tc.tile_pool(name="psum", bufs=1, space="PSUM"))

            def s_tile(name):
                # 2-bank slots shared by score-pairs, chains, out-proj, tr
                return psum.tile([128, 2 * SC], F32, tag="s", bufs=3,
                                 name=name)

            # ---- weights on the scalar (ACT) DGE queue, before any exp ----
            def load_w_all(dram, nm, splits=(8,)):
                t = pw.tile([128, NHC * 256], BF16, tag=nm, name=nm)
                dst = t[:].rearrange("p (c n) -> p c n", c=NHC)
                src = dram.rearrange("(c p) n -> p c n", p=128)
                lo = 0
                for hi in splits:
                    nc.scalar.dma_start(dst[:, lo:hi, :], src[:, lo:hi, :])
                    lo = hi
                return [t[:, hc * 256:(hc + 1) * 256] for hc in range(NHC)]

            wq_t = load_w_all(wq, "wqa", splits=(4, 8))
            wk_t = load_w_all(wk, "wka")
            aux_t = pw.tile([128, 132], BF16, tag="aux")
            nc.scalar.dma_start(aux_t[:], aux[:])
            ident = aux_t[:, 0:128]
            ones1 = aux_t[:, 128:129]
            wv_t = load_w_all(wv, "wva")
            wo_t = []
            for p in range(2):
                t = pw.tile([128, H], BF16, tag=f"wo{p}", name=f"wo{p}")
                nc.scalar.dma_start(t[:], wo[p * 128:(p + 1) * 128, :])
                wo_t.append(t)

            # ---- persistent activations ----
            QT = [pbig.tile([128, S], BF16, tag=f"qt{p}", name=f"qt{p}")
                  for p in range(2)]
            KT = [pbig.tile([128, S], BF16, tag=f"kt{p}", name=f"kt{p}")
                  for p in range(2)]
            VT = [pbig.tile([128, S], BF16, tag=f"vt{p}", name=f"vt{p}")
                  for p in range(2)]
            VA = [pbig.tile([128, 260], BF16, tag=f"va{t_}", name=f"va{t_}")
                  for t_ in range(S // 128)]
            for t_ in range(S // 128):
                ones_cols = VA[t_][:].rearrange(
                    "q (h c) -> q h c", c=65)[:, :, 64]
                nc.gpsimd.memset(ones_cols, 1.0)

            xt_tiles = [None] * NSC

            def load_xt(j, splits):
                xt_all = pxt.tile([128, NHC * SC], BF16, tag="xt",
                                  name=f"xt{j}")
                sj = slice(j * SC, (j + 1) * SC)
                xt_src = xT.rearrange("(c p) s -> p c s", p=128)[:, :, sj]
                xt_dst = xt_all[:].rearrange("p (c s) -> p c s", c=NHC)
                lo = 0
                for hi in splits:
                    nc.sync.dma_start(xt_dst[:, lo:hi, :],
                                      xt_src[:, lo:hi, :])
                    lo = hi
                xt_tiles[j] = xt_all

            # ---- QKV projection chains ----
            on_diag = [False]

            def qk_chain(j, W, OUT, p, tag=None):
                def emit():
                    xt_all = xt_tiles[j]
                    sj = slice(j * SC, (j + 1) * SC)
                    t_ = tag or ("s" if on_diag[0] else "pv")
                    if t_ == "s":
                        ps = s_tile(f"qk{j}_{p}")[:, 0:SC]
                    else:
                        ps = psum.tile([128, SC], F32, tag="pv", bufs=2,
                                       name=f"qk{j}_{p}")
                    for hc in range(NHC):
                        nc.tensor.matmul(
                            ps[:], W[hc][:, p * 128:(p + 1) * 128],
                            xt_all[:, hc * SC:(hc + 1) * SC],
                            start=(hc == 0), stop=(hc == NHC - 1))
                    nc.vector.tensor_copy(OUT[p][:, sj], ps[:])
                return emit

            def v_chain(j, tci):
                def emit():
                    xt_all = xt_tiles[j]
                    t_ = 4 * j + tci
                    if on_diag[0]:
                        ps = s_tile(f"v{j}_{tci}")[:, 0:SC]
                    else:
                        ps = psum.tile([128, SC], F32, tag="pv", bufs=2,
                                       name=f"v{j}_{tci}")
                    for hc in range(NHC):
                        nc.tensor.matmul(
                            ps[:, 0:256],
                            xt_all[:, hc * SC + tci * 128:
                                   hc * SC + (tci + 1) * 128],
                            wv_t[hc], start=(hc == 0), stop=(hc == NHC - 1))
                    dst = VA[t_][:].rearrange(
                        "q (h c) -> q h c", c=65)[:, :, 0:64]
                    nc.vector.tensor_copy(
                        dst, ps[:, 0:256].rearrange(
                            "q (h c) -> q h c", c=64))
                return emit

            # ---- chunk-local state: pt tiles persist per chunk ----
            class ChunkCtx:
                def __init__(self, j):
                    self.pts = {}   # (tcc, h) -> pt tile
                    self.zz = pzz.tile([128, 16], F32, tag="zz",
                                       name=f"zz{j}")

            # ---- attention pieces ----
            def emit_S(cc, j, tcc, prs):
                k = tcc - 4 * j
                c0 = max(0, 128 * k)
                sjv = slice(j * SC + c0, (j + 1) * SC)
                for p in prs:
                    ss = s_tile(f"ss{tcc}_{p}")
                    for r in range(2):
                        nc.tensor.matmul(
                            ss[:, SC * r + c0:SC * (r + 1)],
                            KT[p][64 * r:64 * (r + 1),
                                  tcc * 128:(tcc + 1) * 128],
                            QT[p][64 * r:64 * (r + 1), sjv],
                            start=True, stop=True)
                    pt = ppt.tile([128, 2 * SC], BF16, tag="pt")
                    w2 = SC - c0
                    src2 = ss[:].rearrange("q (r s) -> q r s", r=2)[
                        :, :, c0:SC]
                    dst2 = pt[:].rearrange("q (r s) -> q r s", r=2)[
                        :, :, c0:SC]
                    nc.scalar.activation(dst2, src2, AF.Exp)
                    if k >= 0:
                        band = pt[:].rearrange("q (r s) -> q r s", r=2)[
                            :, :, c0:c0 + 128]
                        nc.gpsimd.affine_select(
                            band, band,
                            pattern=[[0, 2], [1, 128]], base=0,
                            channel_multiplier=-1,
                            compare_op=mybir.AluOpType.is_ge, fill=0.0)
                    cc.pts[(tcc, p)] = pt

            subs = deque()        # deferred out-projection sub-closures
            pe_extras = deque()   # deferred transpose closures

            def make_transpose(cc, j, sti, v2):
                st = 4 * j + sti

                def emit():
                    for p in range(2):
                        trt = s_tile(f"tr{st}_{p}")[:, 0:SC]
                        trp = trt[:, 0:64].bitcast(BF16)
                        nc.tensor.transpose(
                            trp, v2[:, 128 * p:128 * (p + 1)], ident)
                        nc.vector.tensor_copy(
                            VT[p][:, st * 128:(st + 1) * 128], trp)
                    ysb = pyo.tile([128, H], BF16, tag="y", name=f"ysb{st}")
                    for n2 in range(2):
                        subs.append(make_sub(st, n2, ysb))
                return emit

            def make_sub(st, n2, ysb):
                def emit():
                    py_ = psum.tile([128, SC], F32, tag="pv", bufs=2,
                                    name=f"py{st}_{n2}")
                    for p in range(2):
                        nc.tensor.matmul(
                            py_[:], VT[p][:, st * 128:(st + 1) * 128],
                            wo_t[p][:, n2 * 512:(n2 + 1) * 512],
                            start=(p == 0), stop=(p == 1))
                    if st >= 12 and n2 == 1:
                        nc.scalar.copy(
                            ysb[:, n2 * 512:(n2 + 1) * 512], py_[:])
                    else:
                        nc.vector.tensor_copy(
                            ysb[:, n2 * 512:(n2 + 1) * 512], py_[:])
                    nc.sync.dma_start(
                        y[st * 128:(st + 1) * 128, n2 * 512:(n2 + 1) * 512],
                        ysb[:, n2 * 512:(n2 + 1) * 512])
                return emit

            def emit_PV(cc, j, sti):
                # one unbroken accumulation group per (s-tile, head):
                # PV over tcc=0..st, then z over tcc=0..st, sequentially
                # through one psum bank (one open group per bank at a time)
                st = 4 * j + sti
                bank = psum.tile([128, SC], F32, tag="pv", bufs=2,
                                 name=f"pv{st}")
                for h in range(4):
                    p_, r_ = divmod(h, 2)
                    o_ = SC * r_ + sti * 128
                    for tcc in range(st + 1):
                        ptsl = cc.pts[(tcc, p_)][:, o_:o_ + 128]
                        nc.tensor.matmul(
                            bank[:, 65 * h:65 * (h + 1)], ptsl,
                            VA[tcc][:, 65 * h:65 * (h + 1)],
                            start=(tcc == 0), stop=(tcc == st))
                nc.vector.reciprocal(
                    cc.zz[:, 4 * sti:4 * sti + 4],
                    bank[:, 0:260].rearrange(
                        "q (h c) -> q h c", c=65)[:, :, 64])
                v2 = pv2.tile([128, 256], BF16, tag="v2", name=f"v2_{st}")
                for h in range(4):
                    nc.vector.tensor_scalar_mul(
                        v2[:, 64 * h:64 * (h + 1)],
                        bank[:, 65 * h:65 * h + 64],
                        cc.zz[:, 4 * sti + h:4 * sti + h + 1])
                if KDBG and st == 0:
                    nc.sync.dma_start(dbg["d_v2_0"][:], v2[:])
                if KDBG and st == 1:
                    nc.sync.dma_start(dbg["d_zz0"][:], cc.zz[:])
                    nc.sync.dma_start(dbg["d_v2_1"][:], v2[:])
                    dsb = pw.tile([128, 64], F32, tag="dsb")
                    nc.vector.tensor_copy(dsb[:], bank[:, 0:64])
                    nc.sync.dma_start(dbg["d_pv1"][:], dsb[:])
                pe_extras.append(make_transpose(cc, j, sti, v2))

            # ---- global schedule ----
            chains = deque()   # (deadline_step, emit_fn)

            load_xt(0, (2, 4, 6, 8))
            qk_chain(0, wq_t, QT, 0, tag="pv")()
            qk_chain(0, wk_t, KT, 0, tag="s")()
            qk_chain(0, wq_t, QT, 1, tag="s")()
            qk_chain(0, wk_t, KT, 1, tag="s")()
            for tci in range(4):
                chains.append((tci + 1, v_chain(0, tci)))

            pending = None
            g = 0
            for j in range(NSC):
                ntc = 4 * j + 4
                cc = ChunkCtx(j)
                if j + 1 < NSC:
                    load_xt(j + 1, (4, 8))
                    g1 = _gstep(j + 1)
                    for p in range(2):
                        chains.append((g1, qk_chain(j + 1, wq_t, QT, p)))
                    for p in range(2):
                        chains.append((g1 + 4 * (j + 1),
                                       qk_chain(j + 1, wk_t, KT, p)))
                    for tci in range(4):
                        chains.append((g1 + 4 * (j + 1) + tci,
                                       v_chain(j + 1, tci)))
                for tcc in range(ntc):
                    k = tcc - 4 * j
                    on_diag[0] = k >= 0
                    emit_S(cc, j, tcc, [0])
                    while pe_extras:
                        pe_extras.popleft()()
                    if pending is not None:
                        emit_PV(*pending)
                        pending = None
                    emit_S(cc, j, tcc, [1])
                    if k >= 0:
                        pending = (cc, j, k)
                    # non-S psum users this step: <= 1 chain + <= 1 sub
                    popped = 0
                    while chains and (chains[0][0] <= g + 1
                                      or (popped == 0
                                          and chains[0][0] <= g + 6)):
                        chains.popleft()[1]()
                        popped += 1
                        if popped >= 2 and not (chains
                                                and chains[0][0] <= g + 1):
                            break
                    nsub = 2 if (j == 3 and k < 0) else 0
                    for _ in range(min(nsub, len(subs))):
                        subs.popleft()()
                    g += 1
            emit_PV(*pending)
            while pe_extras:
                pe_extras.popleft()()
            while subs:
                subs.popleft()()
            if KDBG:
                nc.sync.dma_start(dbg["d_vt0"][:], VT[0][:])
                nc.sync.dma_start(dbg["d_vt1"][:], VT[1][:])
                nc.sync.dma_start(dbg["d_qt0"][:], QT[0][:])
                nc.sync.dma_start(dbg["d_kt0"][:], KT[0][:])
                nc.sync.dma_start(dbg["d_va0"][:], VA[0][:])
    nc.compile()
    return nc


def _in_maps(x, w_qkv, w_out):
    import ml_dtypes
    bf16 = ml_dtypes.bfloat16
    x = np.asarray(x, dtype=np.float32)
    w_qkv = np.asarray(w_qkv, dtype=np.float32)
    w_out = np.asarray(w_out, dtype=np.float32)
    aux_const = np.zeros((128, 132), dtype=np.float32)
    aux_const[:, 0:128] = np.eye(128, dtype=np.float32)
    aux_const[:, 128] = 1.0
    aux_const = aux_const.astype(bf16)
    scale = np.float32(1.0 / np.sqrt(DH))
    in_maps = []
    for c in range(NCORES):
        b, g = divmod(c, 4)
        cols = slice(256 * g, 256 * (g + 1))
        in_maps.append({
            "xT": np.ascontiguousarray(x[b].T).astype(bf16),
            "wq": (np.ascontiguousarray(w_qkv[:, 0 * H:1 * H][:, cols])
                   * scale).astype(bf16),
            "wk": np.ascontiguousarray(
                w_qkv[:, 1 * H:2 * H][:, cols]).astype(bf16),
            "wv": np.ascontiguousarray(
                w_qkv[:, 2 * H:3 * H][:, cols]).astype(bf16),
            "wo": np.ascontiguousarray(w_out[cols, :]).astype(bf16),
            "aux": aux_const,
        })
    return in_maps


TRACE = False
LAST_RESULTS = None


def kernel(x, w_qkv, w_out):
    global LAST_RESULTS
    if "nc" not in _CACHE:
        _CACHE["nc"] = _build()
    nc = _CACHE["nc"]
    in_maps = _in_maps(x, w_qkv, w_out)
    res = bass_utils.run_bass_kernel_spmd(
        nc, in_maps, core_ids=list(range(NCORES)), trace=TRACE)
    LAST_RESULTS = res
    y = np.zeros((B, S, H), dtype=np.float32)
    for c in range(NCORES):
        y[c // 4] += np.asarray(res.results[c]["y"], dtype=np.float32)
    return y


# revision 48
# speedup vs baseline: 1.3395x; 1.0103x over previous
"""Causal attention block (B=2, S=2048, H=1024, 16 heads) on 8 NeuronCores.

Sharding: core c handles batch b = c // 4 and head-group g = c % 4
(4 heads = 256 qkv columns / w_out rows per core). Each core computes a
partial output y_partial = softmax(QK^T/sqrt(d)) V @ Wout_slice for its
heads; the host sums the 4 head-group partials per batch.

Design (bf16 compute, f32 PSUM):
  x^T [1024, 2048] bf16 per 512-col s-chunk; Q^T,K^T head-pair tiles
  [128, 2048] bf16 (d on partitions); V natural [t, d] per t-chunk in
  VA [128, 260] bf16 with an interleaved ones column per head (cols
  65h+64), so the PV matmul emits softmax denominators for free.
  Scores: S^T = K^T.T @ Q^T per (head-pair, t-chunk, s-chunk) into a
  2-bank psum pair tile; ONE exp per pair (2-region AP) -> persistent
  pt [128, 1024] bf16; diagonal triangles zeroed post-exp by a single
  gpsimd affine_select over both heads (pattern [[0,2],[1,128]]).
  PV flipped to [s, d] orientation: the pt block is the *stationary*
  operand and VA the 65-wide moving operand, so each PV step costs 65
  rows instead of 512. PSUM allows only ONE open accumulation group per
  bank, so each (s-tile, head) accumulates over its whole t-range as one
  unbroken matmul group, batched at diagonal steps (pt tiles persist per
  chunk). Z lands per-partition -> reciprocal + per-partition scalar
  multiply (no broadcast dance), then one PE transpose (identity rhs)
  per head pair into VT [d, s] for the out-projection. Transposes write
  into fresh 2-bank slots only: a transpose clobbers its whole psum bank.
  Schedule: software pipeline with PV batches lagging S/exp one step and
  transposes two; next-chunk QKV chains are deadline-scheduled PE filler;
  all out-projections (half-H sub-units) are deferred into the ACT-bound
  final chunk, with tail ysb copies alternating DVE/ACT.
  PSUM: 3 x 2-bank score slots + 2 rotating banks (PV groups / chains /
  out-proj / transposes).  Queues: x^T + y on sync(SP), weights on
  scalar(ACT) issued before any exp.
"""

import numpy as np
from collections import deque
from contextlib import ExitStack

import concourse.bass as bass
import concourse.tile as tile
import concourse.mybir as mybir
from concourse import bacc
from concourse import bass_utils

F32 = mybir.dt.float32
BF16 = mybir.dt.bfloat16
AF = mybir.ActivationFunctionType

B, S, H = 2, 2048, 1024
NH, DH = 16, 64
NCORES = 8
SC = 512            # s-chunk width
NSC = S // SC       # 4
NHC = H // 128      # 8 h contraction chunks

_CACHE = {}


def _gstep(j):
    return 2 * j * (j + 1)


def _build():
    nc = bacc.Bacc("TRN2", target_bir_lowering=False, debug=False,
                   enable_asserts=False, num_devices=NCORES)
    xT = nc.dram_tensor("xT", [H, S], BF16, kind="ExternalInput").ap()
    wq = nc.dram_tensor("wq", [H, 256], BF16, kind="ExternalInput").ap()
    wk = nc.dram_tensor("wk", [H, 256], BF16, kind="ExternalInput").ap()
    wv = nc.dram_tensor("wv", [H, 256], BF16, kind="ExternalInput").ap()
    wo = nc.dram_tensor("wo", [256, H], BF16, kind="ExternalInput").ap()
    aux = nc.dram_tensor("aux", [128, 132], BF16, kind="ExternalInput").ap()
    y = nc.dram_tensor("y", [S, H], BF16, kind="ExternalOutput").ap()
    import os
    KDBG = os.environ.get("KDBG", "0") == "1"
    if KDBG:
        dbg = {nm: nc.dram_tensor(nm, shp, dt, kind="ExternalOutput").ap()
               for nm, shp, dt in [
                   ("d_vt0", [128, S], BF16), ("d_vt1", [128, S], BF16),
                   ("d_qt0", [128, S], BF16), ("d_kt0", [128, S], BF16),
                   ("d_va0", [128, 256], BF16), ("d_zz0", [128, 16], F32),
                   ("d_v2_0", [128, 256], BF16), ("d_pt00", [128, SC], BF16),
                   ("d_pt01", [128, SC], BF16), ("d_pv1", [128, 64], F32),
                   ("d_v2_1", [128, 256], BF16),
               ]}

    with tile.TileContext(nc) as tc:
        with ExitStack() as ctx:
            pw = ctx.enter_context(tc.tile_pool(name="w", bufs=1))
            pxt = ctx.enter_context(tc.tile_pool(name="xt", bufs=2))
            pbig = ctx.enter_context(tc.tile_pool(name="big", bufs=1))
            ppt = ctx.enter_context(tc.tile_pool(name="pt", bufs=36))
            pzz = ctx.enter_context(tc.tile_pool(name="zz", bufs=2))
            pv2 = ctx.enter_context(tc.tile_pool(name="v2", bufs=3))
            pyo = ctx.enter_context(tc.tile_pool(name="yo", bufs=4))
            psum = ctx.enter_context(
                tc.tile_pool(name="psum", bufs=1, space="PSUM"))

            def s_tile(name):
                # 2-bank slots shared by score-pairs, chains, out-proj, tr
                return psum.tile([128, 2 * SC], F32, tag="s", bufs=3,
                                 name=name)

            # ---- weights on the scalar (ACT) DGE queue, before any exp ----
            def load_w_all(dram, nm, splits=(8,)):
                t = pw.tile([128, NHC * 256], BF16, tag=nm, name=nm)
                dst = t[:].rearrange("p (c n) -> p c n", c=NHC)
                src = dram.rearrange("(c p) n -> p c n", p=128)
                lo = 0
                for hi in splits:
                    nc.scalar.dma_start(dst[:, lo:hi, :], src[:, lo:hi, :])
                    lo = hi
                return [t[:, hc * 256:(hc + 1) * 256] for hc in range(NHC)]

            wq_t = load_w_all(wq, "wqa", splits=(4, 8))
            wk_t = load_w_all(wk, "wka")
            aux_t = pw.tile([128, 132], BF16, tag="aux")
            nc.scalar.dma_start(aux_t[:], aux[:])
            ident = aux_t[:, 0:128]
            ones1 = aux_t[:, 128:129]
            wv_t = load_w_all(wv, "wva")
            wo_t = []
            for p in range(2):
                t = pw.tile([128, H], BF16, tag=f"wo{p}", name=f"wo{p}")
                nc.scalar.dma_start(t[:], wo[p * 128:(p + 1) * 128, :])
                wo_t.append(t)

            # ---- persistent activations ----
            QT = [pbig.tile([128, S], BF16, tag=f"qt{p}", name=f"qt{p}")
                  for p in range(2)]
            KT = [pbig.tile([128, S], BF16, tag=f"kt{p}", name=f"kt{p}")
                  for p in range(2)]
            VT = [pbig.tile([128, S], BF16, tag=f"vt{p}", name=f"vt{p}")
                  for p in range(2)]
            VA = [pbig.tile([128, 260], BF16, tag=f"va{t_}", name=f"va{t_}")
                  for t_ in range(S // 128)]
            for t_ in range(S // 128):
                ones_cols = VA[t_][:].rearrange(
                    "q (h c) -> q h c", c=65)[:, :, 64]
                nc.gpsimd.memset(ones_cols, 1.0)

            xt_tiles = [None] * NSC

            def load_xt(j, splits):
                xt_all = pxt.tile([128, NHC * SC], BF16, tag="xt",
                                  name=f"xt{j}")
                sj = slice(j * SC, (j + 1) * SC)
                xt_src = xT.rearrange("(c p) s -> p c s", p=128)[:, :, sj]
                xt_dst = xt_all[:].rearrange("p (c s) -> p c s", c=NHC)
                lo = 0
                for hi in splits:
                    nc.sync.dma_start(xt_dst[:, lo:hi, :],
                                      xt_src[:, lo:hi, :])
                    lo = hi
                xt_tiles[j] = xt_all

            # ---- QKV projection chains ----
            on_diag = [False]

            def qk_chain(j, W, OUT, p, tag=None):
                def emit():
                    xt_all = xt_tiles[j]
                    sj = slice(j * SC, (j + 1) * SC)
                    t_ = tag or ("s" if on_diag[0] else "pv")
                    if t_ == "s":
                        ps = s_tile(f"qk{j}_{p}")[:, 0:SC]
                    else:
                        ps = psum.tile([128, SC], F32, tag="pv", bufs=2,
                                       name=f"qk{j}_{p}")
                    for hc in range(NHC):
                        nc.tensor.matmul(
                            ps[:], W[hc][:, p * 128:(p + 1) * 128],
                            xt_all[:, hc * SC:(hc + 1) * SC],
                            start=(hc == 0), stop=(hc == NHC - 1))
                    nc.vector.tensor_copy(OUT[p][:, sj], ps[:])
                return emit

            def v_chain(j, tci):
                def emit():
                    xt_all = xt_tiles[j]
                    t_ = 4 * j + tci
                    if on_diag[0]:
                        ps = s_tile(f"v{j}_{tci}")[:, 0:SC]
                    else:
                        ps = psum.tile([128, SC], F32, tag="pv", bufs=2,
                                       name=f"v{j}_{tci}")
                    for hc in range(NHC):
                        nc.tensor.matmul(
                            ps[:, 0:256],
                            xt_all[:, hc * SC + tci * 128:
                                   hc * SC + (tci + 1) * 128],
                            wv_t[hc], start=(hc == 0), stop=(hc == NHC - 1))
                    dst = VA[t_][:].rearrange(
                        "q (h c) -> q h c", c=65)[:, :, 0:64]
                    nc.vector.tensor_copy(
                        dst, ps[:, 0:256].rearrange(
                            "q (h c) -> q h c", c=64))
                return emit

            # ---- chunk-local state: pt tiles persist per chunk ----
            class ChunkCtx:
                def __init__(self, j):
                    self.pts = {}   # (tcc, h) -> pt tile
                    self.zz = pzz.tile([128, 16], F32, tag="zz",
                                       name=f"zz{j}")

            # ---- attention pieces ----
            def emit_S(cc, j, tcc, prs):
                k = tcc - 4 * j
                c0 = max(0, 128 * k)
                sjv = slice(j * SC + c0, (j + 1) * SC)
                for p in prs:
                    ss = s_tile(f"ss{tcc}_{p}")
                    for r in range(2):
                        nc.tensor.matmul(
                            ss[:, SC * r + c0:SC * (r + 1)],
                            KT[p][64 * r:64 * (r + 1),
                                  tcc * 128:(tcc + 1) * 128],
                            QT[p][64 * r:64 * (r + 1), sjv],
                            start=True, stop=True)
                    pt = ppt.tile([128, 2 * SC], BF16, tag="pt")
                    w2 = SC - c0
                    src2 = ss[:].rearrange("q (r s) -> q r s", r=2)[
                        :, :, c0:SC]
                    dst2 = pt[:].rearrange("q (r s) -> q r s", r=2)[
                        :, :, c0:SC]
                    nc.scalar.activation(dst2, src2, AF.Exp)
                    if k >= 0:
                        band = pt[:].rearrange("q (r s) -> q r s", r=2)[
                            :, :, c0:c0 + 128]
                        nc.gpsimd.affine_select(
                            band, band,
                            pattern=[[0, 2], [1, 128]], base=0,
                            channel_multiplier=-1,
                            compare_op=mybir.AluOpType.is_ge, fill=0.0)
                    cc.pts[(tcc, p)] = pt

            subs = deque()        # deferred out-projection sub-closures
            pe_extras = deque()   # deferred transpose closures

            def make_transpose(cc, j, sti, v2):
                st = 4 * j + sti

                def emit():
                    for p in range(2):
                        trt = s_tile(f"tr{st}_{p}")[:, 0:SC]
                        trp = trt[:, 0:64].bitcast(BF16)
                        nc.tensor.transpose(
                            trp, v2[:, 128 * p:128 * (p + 1)], ident)
                        nc.vector.tensor_copy(
                            VT[p][:, st * 128:(st + 1) * 128], trp)
                    ysb = pyo.tile([128, H], BF16, tag="y", name=f"ysb{st}")
                    for n2 in range(2):
                        subs.append(make_sub(st, n2, ysb))
                return emit

            def make_sub(st, n2, ysb):
                def emit():
                    py_ = psum.tile([128, SC], F32, tag="pv", bufs=2,
                                    name=f"py{st}_{n2}")
                    for p in range(2):
                        nc.tensor.matmul(
                            py_[:], VT[p][:, st * 128:(st + 1) * 128],
                            wo_t[p][:, n2 * 512:(n2 + 1) * 512],
                            start=(p == 0), stop=(p == 1))
                    if st >= 12 and n2 == 1:
                        nc.scalar.copy(
                            ysb[:, n2 * 512:(n2 + 1) * 512], py_[:])
                    else:
                        nc.vector.tensor_copy(
                            ysb[:, n2 * 512:(n2 + 1) * 512], py_[:])
                    nc.sync.dma_start(
                        y[st * 128:(st + 1) * 128, n2 * 512:(n2 + 1) * 512],
                        ysb[:, n2 * 512:(n2 + 1) * 512])
                return emit

            def emit_PV(cc, j, sti):
                # one unbroken accumulation group per (s-tile, head):
                # PV over tcc=0..st, then z over tcc=0..st, sequentially
                # through one psum bank (one open group per bank at a time)
                st = 4 * j + sti
                bank = psum.tile([128, SC], F32, tag="pv", bufs=2,
                                 name=f"pv{st}")
                for h in range(4):
                    p_, r_ = divmod(h, 2)
                    o_ = SC * r_ + sti * 128
                    for tcc in range(st + 1):
                        ptsl = cc.pts[(tcc, p_)][:, o_:o_ + 128]
                        nc.tensor.matmul(
                            bank[:, 65 * h:65 * (h + 1)], ptsl,
                            VA[tcc][:, 65 * h:65 * (h + 1)],
                            start=(tcc == 0), stop=(tcc == st))
                nc.vector.reciprocal(
                    cc.zz[:, 4 * sti:4 * sti + 4],
                    bank[:, 0:260].rearrange(
                        "q (h c) -> q h c", c=65)[:, :, 64])
                v2 = pv2.tile([128, 256], BF16, tag="v2", name=f"v2_{st}")
                for h in range(4):
                    nc.vector.tensor_scalar_mul(
                        v2[:, 64 * h:64 * (h + 1)],
                        bank[:, 65 * h:65 * h + 64],
                        cc.zz[:, 4 * sti + h:4 * sti + h + 1])
                if KDBG and st == 0:
                    nc.sync.dma_start(dbg["d_v2_0"][:], v2[:])
                if KDBG and st == 1:
                    nc.sync.dma_start(dbg["d_zz0"][:], cc.zz[:])
                    nc.sync.dma_start(dbg["d_v2_1"][:], v2[:])
                    dsb = pw.tile([128, 64], F32, tag="dsb")
                    nc.vector.tensor_copy(dsb[:], bank[:, 0:64])
                    nc.sync.dma_start(dbg["d_pv1"][:], dsb[:])
                pe_extras.append(make_transpose(cc, j, sti, v2))

            # ---- global schedule ----
            chains = deque()   # (deadline_step, emit_fn)

            load_xt(0, (2, 4, 6, 8))
            qk_chain(0, wq_t, QT, 0, tag="pv")()
            qk_chain(0, wk_t, KT, 0, tag="s")()
            qk_chain(0, wq_t, QT, 1, tag="s")()
            qk_chain(0, wk_t, KT, 1, tag="s")()
            for tci in range(4):
                chains.append((tci + 1, v_chain(0, tci)))

            pending = None
            g = 0
            for j in range(NSC):
                ntc = 4 * j + 4
                cc = ChunkCtx(j)
                if j + 1 < NSC:
                    load_xt(j + 1, (4, 8))
                    g1 = _gstep(j + 1)
                    for p in range(2):
                        chains.append((g1, qk_chain(j + 1, wq_t, QT, p)))
                    for p in range(2):
                        chains.append((g1 + 4 * (j + 1),
                                       qk_chain(j + 1, wk_t, KT, p)))
                    for tci in range(4):
                        chains.append((g1 + 4 * (j + 1) + tci,
                                       v_chain(j + 1, tci)))
                for tcc in range(ntc):
                    k = tcc - 4 * j
                    on_diag[0] = k >= 0
                    emit_S(cc, j, tcc, [0])
                    while pe_extras:
                        pe_extras.popleft()()
                    if pending is not None:
                        emit_PV(*pending)
                        pending = None
                    emit_S(cc, j, tcc, [1])
                    if k >= 0:
                        pending = (cc, j, k)
                    # non-S psum users this step: <= 1 chain + <= 1 sub
                    popped = 0
                    while chains and (chains[0][0] <= g + 1
                                      or (popped == 0
                                          and chains[0][0] <= g + 5)):
                        chains.popleft()[1]()
                        popped += 1
                        if popped >= 2 and not (chains
                                                and chains[0][0] <= g + 1):
                            break
                    nsub = (2 if (j == 3 and k < 0) else
                            (1 if (k < 0 and ((j == 2 and tcc >= 4
                                               and len(subs) > 8)
                                              or (j == 1 and tcc >= 2
                                                  and len(subs) > 4)))
                             else 0))
                    for _ in range(min(nsub, len(subs))):
                        subs.popleft()()
                    g += 1
            emit_PV(*pending)
            while pe_extras:
                pe_extras.popleft()()
            while subs:
                subs.popleft()()
            if KDBG:
                nc.sync.dma_start(dbg["d_vt0"][:], VT[0][:])
                nc.sync.dma_start(dbg["d_vt1"][:], VT[1][:])
                nc.sync.dma_start(dbg["d_qt0"][:], QT[0][:])
                nc.sync.dma_start(dbg["d_kt0"][:], KT[0][:])
                nc.sync.dma_start(dbg["d_va0"][:], VA[0][:])
    nc.compile()
    return nc


def _in_maps(x, w_qkv, w_out):
    import ml_dtypes
    bf16 = ml_dtypes.bfloat16
    x = np.asarray(x, dtype=np.float32)
    w_qkv = np.asarray(w_qkv, dtype=np.float32)
    w_out = np.asarray(w_out, dtype=np.float32)
    aux_const = np.zeros((128, 132), dtype=np.float32)
    aux_const[:, 0:128] = np.eye(128, dtype=np.float32)
    aux_const[:, 128] = 1.0
    aux_const = aux_const.astype(bf16)
    scale = np.float32(1.0 / np.sqrt(DH))
    in_maps = []
    for c in range(NCORES):
        b, g = divmod(c, 4)
        cols = slice(256 * g, 256 * (g + 1))
        in_maps.append({
            "xT": np.ascontiguousarray(x[b].T).astype(bf16),
            "wq": (np.ascontiguousarray(w_qkv[:, 0 * H:1 * H][:, cols])
                   * scale).astype(bf16),
            "wk": np.ascontiguousarray(
                w_qkv[:, 1 * H:2 * H][:, cols]).astype(bf16),
            "wv": np.ascontiguousarray(
                w_qkv[:, 2 * H:3 * H][:, cols]).astype(bf16),
            "wo": np.ascontiguousarray(w_out[cols, :]).astype(bf16),
            "aux": aux_const,
        })
    return in_maps


TRACE = False
LAST_RESULTS = None


def kernel(x, w_qkv, w_out):
    global LAST_RESULTS
    if "nc" not in _CACHE:
        _CACHE["nc"] = _build()
    nc = _CACHE["nc"]
    in_maps = _in_maps(x, w_qkv, w_out)
    res = bass_utils.run_bass_kernel_spmd(
        nc, in_maps, core_ids=list(range(NCORES)), trace=TRACE)
    LAST_RESULTS = res
    y = np.zeros((B, S, H), dtype=np.float32)
    for c in range(NCORES):
        y[c // 4] += np.asarray(res.results[c]["y"], dtype=np.float32)
    return y


# revision 49
# speedup vs baseline: 1.3450x; 1.0040x over previous
"""Causal attention block (B=2, S=2048, H=1024, 16 heads) on 8 NeuronCores.

Sharding: core c handles batch b = c // 4 and head-group g = c % 4
(4 heads = 256 qkv columns / w_out rows per core). Each core computes a
partial output y_partial = softmax(QK^T/sqrt(d)) V @ Wout_slice for its
heads; the host sums the 4 head-group partials per batch.

Design (bf16 compute, f32 PSUM):
  x^T [1024, 2048] bf16 per 512-col s-chunk; Q^T,K^T head-pair tiles
  [128, 2048] bf16 (d on partitions); V natural [t, d] per t-chunk in
  VA [128, 260] bf16 with an interleaved ones column per head (cols
  65h+64), so the PV matmul emits softmax denominators for free.
  Scores: S^T = K^T.T @ Q^T per (head-pair, t-chunk, s-chunk) into a
  2-bank psum pair tile; ONE exp per pair (2-region AP) -> persistent
  pt [128, 1024] bf16; diagonal triangles zeroed post-exp by a single
  gpsimd affine_select over both heads (pattern [[0,2],[1,128]]).
  PV flipped to [s, d] orientation: the pt block is the *stationary*
  operand and VA the 65-wide moving operand, so each PV step costs 65
  rows instead of 512. PSUM allows only ONE open accumulation group per
  bank, so each (s-tile, head) accumulates over its whole t-range as one
  unbroken matmul group, batched at diagonal steps (pt tiles persist per
  chunk). Z lands per-partition -> reciprocal + per-partition scalar
  multiply (no broadcast dance), then one PE transpose (identity rhs)
  per head pair into VT [d, s] for the out-projection. Transposes write
  into fresh 2-bank slots only: a transpose clobbers its whole psum bank.
  Schedule: software pipeline with PV batches lagging S/exp one step and
  transposes two; next-chunk QKV chains are deadline-scheduled PE filler;
  all out-projections (half-H sub-units) are deferred into the ACT-bound
  final chunk, with tail ysb copies alternating DVE/ACT.
  PSUM: 3 x 2-bank score slots + 2 rotating banks (PV groups / chains /
  out-proj / transposes).  Queues: x^T + y on sync(SP), weights on
  scalar(ACT) issued before any exp.
"""

import numpy as np
from collections import deque
from contextlib import ExitStack

import concourse.bass as bass
import concourse.tile as tile
import concourse.mybir as mybir
from concourse import bacc
from concourse import bass_utils

F32 = mybir.dt.float32
BF16 = mybir.dt.bfloat16
AF = mybir.ActivationFunctionType

B, S, H = 2, 2048, 1024
NH, DH = 16, 64
NCORES = 8
SC = 512            # s-chunk width
NSC = S // SC       # 4
NHC = H // 128      # 8 h contraction chunks

_CACHE = {}


def _gstep(j):
    return 2 * j * (j + 1)


def _build():
    nc = bacc.Bacc("TRN2", target_bir_lowering=False, debug=False,
                   enable_asserts=False, num_devices=NCORES)
    xT = nc.dram_tensor("xT", [H, S], BF16, kind="ExternalInput").ap()
    wq = nc.dram_tensor("wq", [H, 256], BF16, kind="ExternalInput").ap()
    wk = nc.dram_tensor("wk", [H, 256], BF16, kind="ExternalInput").ap()
    wv = nc.dram_tensor("wv", [H, 256], BF16, kind="ExternalInput").ap()
    wo = nc.dram_tensor("wo", [256, H], BF16, kind="ExternalInput").ap()
    aux = nc.dram_tensor("aux", [128, 132], BF16, kind="ExternalInput").ap()
    y = nc.dram_tensor("y", [S, H], BF16, kind="ExternalOutput").ap()
    import os
    KDBG = os.environ.get("KDBG", "0") == "1"
    if KDBG:
        dbg = {nm: nc.dram_tensor(nm, shp, dt, kind="ExternalOutput").ap()
               for nm, shp, dt in [
                   ("d_vt0", [128, S], BF16), ("d_vt1", [128, S], BF16),
                   ("d_qt0", [128, S], BF16), ("d_kt0", [128, S], BF16),
                   ("d_va0", [128, 256], BF16), ("d_zz0", [128, 16], F32),
                   ("d_v2_0", [128, 256], BF16), ("d_pt00", [128, SC], BF16),
                   ("d_pt01", [128, SC], BF16), ("d_pv1", [128, 64], F32),
                   ("d_v2_1", [128, 256], BF16),
               ]}

    with tile.TileContext(nc) as tc:
        with ExitStack() as ctx:
            pw = ctx.enter_context(tc.tile_pool(name="w", bufs=1))
            pxt = ctx.enter_context(tc.tile_pool(name="xt", bufs=2))
            pbig = ctx.enter_context(tc.tile_pool(name="big", bufs=1))
            ppt = ctx.enter_context(tc.tile_pool(name="pt", bufs=36))
            pzz = ctx.enter_context(tc.tile_pool(name="zz", bufs=2))
            pv2 = ctx.enter_context(tc.tile_pool(name="v2", bufs=3))
            pyo = ctx.enter_context(tc.tile_pool(name="yo", bufs=4))
            psum = ctx.enter_context(
                tc.tile_pool(name="psum", bufs=1, space="PSUM"))

            def s_tile(name):
                # 2-bank slots shared by score-pairs, chains, out-proj, tr
                return psum.tile([128, 2 * SC], F32, tag="s", bufs=3,
                                 name=name)

            # ---- weights on the scalar (ACT) DGE queue, before any exp ----
            def load_w_all(dram, nm, splits=(8,)):
                t = pw.tile([128, NHC * 256], BF16, tag=nm, name=nm)
                dst = t[:].rearrange("p (c n) -> p c n", c=NHC)
                src = dram.rearrange("(c p) n -> p c n", p=128)
                lo = 0
                for hi in splits:
                    nc.scalar.dma_start(dst[:, lo:hi, :], src[:, lo:hi, :])
                    lo = hi
                return [t[:, hc * 256:(hc + 1) * 256] for hc in range(NHC)]

            wq_t = load_w_all(wq, "wqa", splits=(4, 8))
            wk_t = load_w_all(wk, "wka")
            aux_t = pw.tile([128, 132], BF16, tag="aux")
            nc.scalar.dma_start(aux_t[:], aux[:])
            ident = aux_t[:, 0:128]
            ones1 = aux_t[:, 128:129]
            wv_t = load_w_all(wv, "wva")
            wo_t = []
            for p in range(2):
                t = pw.tile([128, H], BF16, tag=f"wo{p}", name=f"wo{p}")
                nc.scalar.dma_start(t[:], wo[p * 128:(p + 1) * 128, :])
                wo_t.append(t)

            # ---- persistent activations ----
            QT = [pbig.tile([128, S], BF16, tag=f"qt{p}", name=f"qt{p}")
                  for p in range(2)]
            KT = [pbig.tile([128, S], BF16, tag=f"kt{p}", name=f"kt{p}")
                  for p in range(2)]
            VT = [pbig.tile([128, S], BF16, tag=f"vt{p}", name=f"vt{p}")
                  for p in range(2)]
            VA = [pbig.tile([128, 260], BF16, tag=f"va{t_}", name=f"va{t_}")
                  for t_ in range(S // 128)]
            for t_ in range(S // 128):
                ones_cols = VA[t_][:].rearrange(
                    "q (h c) -> q h c", c=65)[:, :, 64]
                nc.gpsimd.memset(ones_cols, 1.0)

            xt_tiles = [None] * NSC

            def load_xt(j, splits):
                xt_all = pxt.tile([128, NHC * SC], BF16, tag="xt",
                                  name=f"xt{j}")
                sj = slice(j * SC, (j + 1) * SC)
                xt_src = xT.rearrange("(c p) s -> p c s", p=128)[:, :, sj]
                xt_dst = xt_all[:].rearrange("p (c s) -> p c s", c=NHC)
                lo = 0
                for hi in splits:
                    nc.sync.dma_start(xt_dst[:, lo:hi, :],
                                      xt_src[:, lo:hi, :])
                    lo = hi
                xt_tiles[j] = xt_all

            # ---- QKV projection chains ----
            on_diag = [False]

            def qk_chain(j, W, OUT, p, tag=None):
                def emit():
                    xt_all = xt_tiles[j]
                    sj = slice(j * SC, (j + 1) * SC)
                    t_ = tag or ("s" if on_diag[0] else "pv")
                    if t_ == "s":
                        ps = s_tile(f"qk{j}_{p}")[:, 0:SC]
                    else:
                        ps = psum.tile([128, SC], F32, tag="pv", bufs=2,
                                       name=f"qk{j}_{p}")
                    for hc in range(NHC):
                        nc.tensor.matmul(
                            ps[:], W[hc][:, p * 128:(p + 1) * 128],
                            xt_all[:, hc * SC:(hc + 1) * SC],
                            start=(hc == 0), stop=(hc == NHC - 1))
                    nc.vector.tensor_copy(OUT[p][:, sj], ps[:])
                return emit

            def v_chain(j, tci):
                def emit():
                    xt_all = xt_tiles[j]
                    t_ = 4 * j + tci
                    if on_diag[0]:
                        ps = s_tile(f"v{j}_{tci}")[:, 0:SC]
                    else:
                        ps = psum.tile([128, SC], F32, tag="pv", bufs=2,
                                       name=f"v{j}_{tci}")
                    for hc in range(NHC):
                        nc.tensor.matmul(
                            ps[:, 0:256],
                            xt_all[:, hc * SC + tci * 128:
                                   hc * SC + (tci + 1) * 128],
                            wv_t[hc], start=(hc == 0), stop=(hc == NHC - 1))
                    dst = VA[t_][:].rearrange(
                        "q (h c) -> q h c", c=65)[:, :, 0:64]
                    nc.vector.tensor_copy(
                        dst, ps[:, 0:256].rearrange(
                            "q (h c) -> q h c", c=64))
                return emit

            # ---- chunk-local state: pt tiles persist per chunk ----
            class ChunkCtx:
                def __init__(self, j):
                    self.pts = {}   # (tcc, h) -> pt tile
                    self.zz = pzz.tile([128, 16], F32, tag="zz",
                                       name=f"zz{j}")

            # ---- attention pieces ----
            def emit_S(cc, j, tcc, prs):
                k = tcc - 4 * j
                c0 = max(0, 128 * k)
                sjv = slice(j * SC + c0, (j + 1) * SC)
                for p in prs:
                    ss = s_tile(f"ss{tcc}_{p}")
                    for r in range(2):
                        nc.tensor.matmul(
                            ss[:, SC * r + c0:SC * (r + 1)],
                            KT[p][64 * r:64 * (r + 1),
                                  tcc * 128:(tcc + 1) * 128],
                            QT[p][64 * r:64 * (r + 1), sjv],
                            start=True, stop=True)
                    pt = ppt.tile([128, 2 * SC], BF16, tag="pt")
                    w2 = SC - c0
                    src2 = ss[:].rearrange("q (r s) -> q r s", r=2)[
                        :, :, c0:SC]
                    dst2 = pt[:].rearrange("q (r s) -> q r s", r=2)[
                        :, :, c0:SC]
                    nc.scalar.activation(dst2, src2, AF.Exp)
                    if k >= 0:
                        band = pt[:].rearrange("q (r s) -> q r s", r=2)[
                            :, :, c0:c0 + 128]
                        nc.gpsimd.affine_select(
                            band, band,
                            pattern=[[0, 2], [1, 128]], base=0,
                            channel_multiplier=-1,
                            compare_op=mybir.AluOpType.is_ge, fill=0.0)
                    cc.pts[(tcc, p)] = pt

            subs = deque()        # deferred out-projection sub-closures
            pe_extras = deque()   # deferred transpose closures

            def make_transpose(cc, j, sti, v2):
                st = 4 * j + sti

                def emit():
                    for p in range(2):
                        trt = s_tile(f"tr{st}_{p}")[:, 0:SC]
                        trp = trt[:, 0:64].bitcast(BF16)
                        nc.tensor.transpose(
                            trp, v2[:, 128 * p:128 * (p + 1)], ident)
                        nc.vector.tensor_copy(
                            VT[p][:, st * 128:(st + 1) * 128], trp)
                    ysb = pyo.tile([128, H], BF16, tag="y", name=f"ysb{st}")
                    for n2 in range(2):
                        subs.append(make_sub(st, n2, ysb))
                return emit

            def make_sub(st, n2, ysb):
                def emit():
                    py_ = psum.tile([128, SC], F32, tag="pv", bufs=2,
                                    name=f"py{st}_{n2}")
                    for p in range(2):
                        nc.tensor.matmul(
                            py_[:], VT[p][:, st * 128:(st + 1) * 128],
                            wo_t[p][:, n2 * 512:(n2 + 1) * 512],
                            start=(p == 0), stop=(p == 1))
                    if st >= 12 and n2 == 1:
                        nc.scalar.copy(
                            ysb[:, n2 * 512:(n2 + 1) * 512], py_[:])
                    else:
                        nc.vector.tensor_copy(
                            ysb[:, n2 * 512:(n2 + 1) * 512], py_[:])
                    nc.sync.dma_start(
                        y[st * 128:(st + 1) * 128, n2 * 512:(n2 + 1) * 512],
                        ysb[:, n2 * 512:(n2 + 1) * 512])
                return emit

            def emit_PV(cc, j, sti):
                # one unbroken accumulation group per (s-tile, head):
                # PV over tcc=0..st, then z over tcc=0..st, sequentially
                # through one psum bank (one open group per bank at a time)
                st = 4 * j + sti
                bank = psum.tile([128, SC], F32, tag="pv", bufs=2,
                                 name=f"pv{st}")
                for h in range(4):
                    p_, r_ = divmod(h, 2)
                    o_ = SC * r_ + sti * 128
                    for tcc in range(st + 1):
                        ptsl = cc.pts[(tcc, p_)][:, o_:o_ + 128]
                        nc.tensor.matmul(
                            bank[:, 65 * h:65 * (h + 1)], ptsl,
                            VA[tcc][:, 65 * h:65 * (h + 1)],
                            start=(tcc == 0), stop=(tcc == st))
                nc.vector.reciprocal(
                    cc.zz[:, 4 * sti:4 * sti + 4],
                    bank[:, 0:260].rearrange(
                        "q (h c) -> q h c", c=65)[:, :, 64])
                v2 = pv2.tile([128, 256], BF16, tag="v2", name=f"v2_{st}")
                for h in range(4):
                    nc.vector.tensor_scalar_mul(
                        v2[:, 64 * h:64 * (h + 1)],
                        bank[:, 65 * h:65 * h + 64],
                        cc.zz[:, 4 * sti + h:4 * sti + h + 1])
                if KDBG and st == 0:
                    nc.sync.dma_start(dbg["d_v2_0"][:], v2[:])
                if KDBG and st == 1:
                    nc.sync.dma_start(dbg["d_zz0"][:], cc.zz[:])
                    nc.sync.dma_start(dbg["d_v2_1"][:], v2[:])
                    dsb = pw.tile([128, 64], F32, tag="dsb")
                    nc.vector.tensor_copy(dsb[:], bank[:, 0:64])
                    nc.sync.dma_start(dbg["d_pv1"][:], dsb[:])
                pe_extras.append(make_transpose(cc, j, sti, v2))

            # ---- global schedule ----
            chains = deque()   # (deadline_step, emit_fn)

            load_xt(0, (2, 4, 6, 8))
            qk_chain(0, wq_t, QT, 0, tag="pv")()
            qk_chain(0, wk_t, KT, 0, tag="s")()
            qk_chain(0, wq_t, QT, 1, tag="s")()
            qk_chain(0, wk_t, KT, 1, tag="s")()
            for tci in range(4):
                chains.append((tci + 1, v_chain(0, tci)))

            pending = None
            g = 0
            for j in range(NSC):
                ntc = 4 * j + 4
                cc = ChunkCtx(j)
                if j + 1 < NSC:
                    load_xt(j + 1, (4, 8))
                    g1 = _gstep(j + 1)
                    for p in range(2):
                        chains.append((g1, qk_chain(j + 1, wq_t, QT, p)))
                    for p in range(2):
                        chains.append((g1 + 4 * (j + 1),
                                       qk_chain(j + 1, wk_t, KT, p)))
                    for tci in range(4):
                        chains.append((g1 + 4 * (j + 1) + tci,
                                       v_chain(j + 1, tci)))
                for tcc in range(ntc):
                    k = tcc - 4 * j
                    on_diag[0] = k >= 0
                    emit_S(cc, j, tcc, [0])
                    while pe_extras:
                        pe_extras.popleft()()
                    if pending is not None:
                        emit_PV(*pending)
                        pending = None
                    emit_S(cc, j, tcc, [1])
                    if k >= 0:
                        pending = (cc, j, k)
                    # non-S psum users this step: <= 1 chain + <= 1 sub
                    popped = 0
                    while chains and (chains[0][0] <= g + 1
                                      or (popped == 0
                                          and chains[0][0] <= g + 5)):
                        chains.popleft()[1]()
                        popped += 1
                        if popped >= 2 and not (chains
                                                and chains[0][0] <= g + 1):
                            break
                    nsub = (2 if (j == 3 and k < 0) else
                            (1 if (k < 0 and ((j == 2 and tcc >= 2
                                               and len(subs) > 8)
                                              or (j == 1 and tcc >= 2
                                                  and len(subs) > 4)))
                             else 0))
                    for _ in range(min(nsub, len(subs))):
                        subs.popleft()()
                    g += 1
            emit_PV(*pending)
            while pe_extras:
                pe_extras.popleft()()
            while subs:
                subs.popleft()()
            if KDBG:
                nc.sync.dma_start(dbg["d_vt0"][:], VT[0][:])
                nc.sync.dma_start(dbg["d_vt1"][:], VT[1][:])
                nc.sync.dma_start(dbg["d_qt0"][:], QT[0][:])
                nc.sync.dma_start(dbg["d_kt0"][:], KT[0][:])
                nc.sync.dma_start(dbg["d_va0"][:], VA[0][:])
    nc.compile()
    return nc


def _in_maps(x, w_qkv, w_out):
    import ml_dtypes
    bf16 = ml_dtypes.bfloat16
    x = np.asarray(x, dtype=np.float32)
    w_qkv = np.asarray(w_qkv, dtype=np.float32)
    w_out = np.asarray(w_out, dtype=np.float32)
    aux_const = np.zeros((128, 132), dtype=np.float32)
    aux_const[:, 0:128] = np.eye(128, dtype=np.float32)
    aux_const[:, 128] = 1.0
    aux_const = aux_const.astype(bf16)
    scale = np.float32(1.0 / np.sqrt(DH))
    in_maps = []
    for c in range(NCORES):
        b, g = divmod(c, 4)
        cols = slice(256 * g, 256 * (g + 1))
        in_maps.append({
            "xT": np.ascontiguousarray(x[b].T).astype(bf16),
            "wq": (np.ascontiguousarray(w_qkv[:, 0 * H:1 * H][:, cols])
                   * scale).astype(bf16),
            "wk": np.ascontiguousarray(
                w_qkv[:, 1 * H:2 * H][:, cols]).astype(bf16),
            "wv": np.ascontiguousarray(
                w_qkv[:, 2 * H:3 * H][:, cols]).astype(bf16),
            "wo": np.ascontiguousarray(w_out[cols, :]).astype(bf16),
            "aux": aux_const,
        })
    return in_maps


TRACE = False
LAST_RESULTS = None


def kernel(x, w_qkv, w_out):
    global LAST_RESULTS
    if "nc" not in _CACHE:
        _CACHE["nc"] = _build()
    nc = _CACHE["nc"]
    in_maps = _in_maps(x, w_qkv, w_out)
    res = bass_utils.run_bass_kernel_spmd(
        nc, in_maps, core_ids=list(range(NCORES)), trace=TRACE)
    LAST_RESULTS = res
    y = np.zeros((B, S, H), dtype=np.float32)
    for c in range(NCORES):
        y[c // 4] += np.asarray(res.results[c]["y"], dtype=np.float32)
    return y
